# revision 1
# baseline (speedup 1.0000x reference)
"""Trainium2 Bass kernel for CTUNOBlock1D (spectral conv + time conv + batchnorm + relu).

Strategy (data-parallel over batch, 8 cores, 4 batches/core):
  - rfft/irfft use only 33 modes -> implemented as DFT matmuls against small
    trig tables (bf16, f32 accumulation on PE).
  - forward pass per batch also accumulates the Gram matrix x^T x (shares the
    loaded weights), which lets BN statistics be computed exactly in mode
    space (Parseval) long before the output tiles exist. The tiny (64,2)
    stats AllReduce therefore overlaps the entire inverse/residual phase.
  - residual branch is folded on host: E_b = K diag(w_t) Wt^T, e_b bias; the
    device computes out^T = Z^T @ ABt + E_b^T @ x^T per batch (channel-major),
    then one scalar-engine pass applies BN scale/shift (+ folded e_b) + ReLU.
  - output is written channel-major and transposed on host.
"""

import os
import numpy as np

import concourse.bass as bass
import concourse.mybir as mybir
import concourse.bacc as bacc
import concourse.tile as tile
from concourse import bass_utils

F32 = mybir.dt.float32
BF16 = mybir.dt.bfloat16
NP_BF16 = mybir.dt.np(BF16)

B, L, CIN, COUT, TEMB = 32, 8192, 64, 64, 256
M = 33            # retained rfft modes
KC = 2 * M        # 66 (real|imag concat)
NCORES = 8
BLOC = B // NCORES   # 4 batches per core
EPS = 1e-5
NCHUNK = L // 128    # 64 l-chunks of 128
STACK = bool(int(os.environ.get("KBENCH_STACK", "1")))
USE_AR = bool(int(os.environ.get("KBENCH_AR", "1")))
PH = int(os.environ.get("KBENCH_PH", "5"))


def _build():
    nc = bacc.Bacc(None, target_bir_lowering=False)

    xb_d = nc.dram_tensor("xb", [BLOC, 128, NCHUNK * CIN], BF16, kind="ExternalInput")
    xt_d = nc.dram_tensor("xt", [BLOC, CIN, L], BF16, kind="ExternalInput")
    cst_d = nc.dram_tensor("cst", [128, NCHUNK * KC], BF16, kind="ExternalInput")
    abt_d = nc.dram_tensor("abt", [CIN, L], BF16, kind="ExternalInput")
    wm_d = nc.dram_tensor("wm", [CIN, M * 128], BF16, kind="ExternalInput")
    ebf_d = nc.dram_tensor("ebf", [CIN, BLOC * COUT], BF16, kind="ExternalInput")
    ef_d = nc.dram_tensor("ef", [CIN, BLOC * COUT], F32, kind="ExternalInput")
    tm_d = nc.dram_tensor("tm", [COUT, 2 * 4 * M], F32, kind="ExternalInput")
    e4_d = nc.dram_tensor("e4", [COUT, BLOC], F32, kind="ExternalInput")
    ep_d = nc.dram_tensor("ep", [128, 2], F32, kind="ExternalInput")
    bnp_d = nc.dram_tensor("bnp", [128, 2], F32, kind="ExternalInput")
    id_d = nc.dram_tensor("idm", [64, 64], F32, kind="ExternalInput")
    out_d = nc.dram_tensor("out", [BLOC, COUT, L], BF16, kind="ExternalOutput")

    with tile.TileContext(nc) as tc:
        with (
            tc.tile_pool(name="const", bufs=1) as cpool,
            tc.tile_pool(name="xs", bufs=1) as xpool,
            tc.tile_pool(name="xtp", bufs=1) as xtpool,
            tc.tile_pool(name="outb", bufs=1) as opool,
            tc.tile_pool(name="small", bufs=2) as spool,
            tc.tile_pool(name="psA", bufs=2, space=bass.MemorySpace.PSUM) as psA,
            tc.tile_pool(name="psS", bufs=2, space=bass.MemorySpace.PSUM) as psS,
            tc.tile_pool(name="psB", bufs=2, space=bass.MemorySpace.PSUM) as psB,
            tc.tile_pool(name="dram", bufs=1, space=bass.MemorySpace.DRAM) as dpool,
        ):
            dma = nc.sync.dma_start

            # ---- critical-path loads first: DFT table + the 4 x tiles ----
            cst_s = cpool.tile([128, NCHUNK * KC], BF16)
            dma(cst_s[:], cst_d[:])
            xss = []
            for b in range(BLOC):
                xs = xpool.tile([128, NCHUNK * CIN], BF16, tag=f"xs{b}")
                dma(xs[:], xb_d[b])
                xss.append(xs)

            # remaining small constants
            wm_s = cpool.tile([CIN, M * 128], BF16)
            tm_s = cpool.tile([COUT, 2 * 4 * M], F32)
            e4_s = cpool.tile([COUT, BLOC], F32)
            ep_s = cpool.tile([128, 2], F32)
            bnp_s = cpool.tile([128, 2], F32)
            id_s = cpool.tile([64, 64], F32)
            ones_s = cpool.tile([64, 1], F32)
            dma(wm_s[:], wm_d[:])
            dma(tm_s[:], tm_d[:])
            dma(e4_s[:], e4_d[:])
            dma(ep_s[:], ep_d[:])
            dma(bnp_s[:], bnp_d[:])
            dma(id_s[:], id_d[:])
            nc.vector.memset(ones_s[:], 1.0)

            ebf_s = cpool.tile([CIN, BLOC * COUT], BF16)   # [i, 64b+o]
            ef_s = cpool.tile([CIN, BLOC * COUT], F32)
            zeb = cpool.tile([128, BLOC * COUT], BF16)     # [0:64]=Z^T, [64:128]=E
            dma(ebf_s[:], ebf_d[:])
            dma(ef_s[:], ef_d[:])
            dma(zeb[64:128, :], ebf_d[:])

            # early dummy Sqrt to pre-load the ACT table set
            warm = spool.tile([1, 1], F32)
            nc.vector.memset(warm[:], 1.0)
            nc.scalar.activation(warm[:], warm[:], mybir.ActivationFunctionType.Sqrt)

            # bulk phase-C inputs, gated behind the critical xs loads: the
            # dummy read of xs3 makes the sync ring wait for xs DMAs before
            # issuing these transfers (keeps HBM bandwidth on the fwd path).
            gate = spool.tile([1, 2], BF16, tag="gate")
            dma(gate[:], xss[BLOC - 1][0:1, 0:2])
            xtc = []
            for b in range(BLOC):
                xt = xtpool.tile([128, L], BF16, tag=f"xtc{b}")
                xtc.append(xt)
            dma(xtc[0][0:64, :], abt_d[:])   # ABt table shares xtc[0]'s top half
            for b in range(BLOC):
                dma(xtc[b][64:128, :], xt_d[b])

            # ---- phase A: forward DFT + Gram per batch ----
            Xsb = cpool.tile([CIN, BLOC * KC], BF16)    # [c, 66b+kcat]
            Gsb = cpool.tile([CIN, BLOC * CIN], BF16)   # [c, 64b+c']
            for b in range(BLOC):
                xs = xss[b]
                xacc = psA.tile([CIN, KC], F32, tag="xacc")
                gacc = psA.tile([CIN, CIN], F32, tag="gacc")
                for u in range(NCHUNK):
                    lhs = xs[:, CIN * u:CIN * (u + 1)]
                    nc.tensor.matmul(xacc[:], lhs, cst_s[:, KC * u:KC * (u + 1)],
                                     start=(u == 0), stop=(u == NCHUNK - 1))
                    nc.tensor.matmul(gacc[:], lhs, lhs,
                                     start=(u == 0), stop=(u == NCHUNK - 1))
                nc.vector.tensor_copy(Xsb[:, KC * b:KC * (b + 1)], xacc[:])
                nc.vector.tensor_copy(Gsb[:, CIN * b:CIN * (b + 1)], gacc[:])

            # ---- phase B: mode mixing ----
            # P1[o, 8k+4j+b] = Wr[k]^T @ X_j ; P2 likewise for Wi (j=0: Xr, 1: Xi)
            P1 = psS.tile([COUT, M * 8], F32, tag="small")
            P2 = psS.tile([COUT, M * 8], F32, tag="small")
            Xr4 = Xsb[:].rearrange("p (b j k) -> p j b k", b=BLOC, j=2, k=M)
            for k in range(M):
                nc.tensor.matmul(P1[:, 8 * k:8 * (k + 1)],
                                 wm_s[:, 128 * k:128 * k + 64],
                                 Xr4[:, :, :, k], start=True, stop=True)
                nc.tensor.matmul(P2[:, 8 * k:8 * (k + 1)],
                                 wm_s[:, 128 * k + 64:128 * (k + 1)],
                                 Xr4[:, :, :, k], start=True, stop=True)

            TT = nc.vector.tensor_tensor
            TS = nc.vector.tensor_scalar
            OP = mybir.AluOpType
            Psb = spool.tile([COUT, 2 * M * 8], F32, tag="psb")
            nc.scalar.copy(Psb[:, 0:M * 8], P1[:])
            nc.scalar.copy(Psb[:, M * 8:2 * M * 8], P2[:])
            Yr = spool.tile([COUT, 4 * M], F32, tag="yr")   # [(k,b)] = 4k+b
            Yi = spool.tile([COUT, 4 * M], F32, tag="yi")
            Pk1 = Psb[:, 0:M * 8].rearrange("p (k x) -> p k x", k=M, x=8)
            Pk2 = Psb[:, M * 8:2 * M * 8].rearrange("p (k x) -> p k x", k=M, x=8)
            Yrv = Yr[:].rearrange("p (k b) -> p k b", k=M, b=4)
            Yiv = Yi[:].rearrange("p (k b) -> p k b", k=M, b=4)
            TT(Yrv, Pk1[:, :, 0:4], Pk2[:, :, 4:8], OP.subtract)
            TT(Yiv, Pk2[:, :, 0:4], Pk1[:, :, 4:8], OP.add)

            Zsb = cpool.tile([COUT, 2 * 4 * M], F32)  # [(ri,k,b)] = 132ri+4k+b
            t1 = spool.tile([COUT, 4 * M], F32, tag="t1")
            t2 = spool.tile([COUT, 4 * M], F32, tag="t2")
            ntm = 4 * M
            TT(t1[:], Yr[:], tm_s[:, 0:ntm], OP.mult)
            TT(t2[:], Yi[:], tm_s[:, ntm:2 * ntm], OP.mult)
            TT(Zsb[:, 0:ntm], t1[:], t2[:], OP.subtract)
            TT(t1[:], Yi[:], tm_s[:, 0:ntm], OP.mult)
            TT(t2[:], Yr[:], tm_s[:, ntm:2 * ntm], OP.mult)
            TT(Zsb[:, ntm:2 * ntm], t1[:], t2[:], OP.add)

            # ---- Z transpose (per batch, DC row folded into bias later) ----
            # zeb[b]: rows 0:64 = Z^T (modes k=1..32, re|im), rows 64:128 = E_b
            Zview = Zsb[:].rearrange("p (ri k b) -> p ri k b", ri=2, k=M, b=4)
            Zflat = spool.tile([COUT, 4 * 64], F32, tag="zflat")  # [b][ri,k>=1]
            nc.vector.tensor_copy(
                Zflat[:].rearrange("p (b ri k) -> p b ri k", b=4, ri=2, k=M - 1),
                Zsb[:].rearrange("p (ri k b) -> p b ri k", ri=2, k=M, b=4)[:, :, :, 1:M])
            for b in range(BLOC):
                tp = psS.tile([CIN, COUT], F32, tag="small")
                nc.tensor.transpose(tp[:], Zflat[:, 64 * b:64 * (b + 1)], id_s[:])
                nc.vector.tensor_copy(zeb[0:64, 64 * b:64 * (b + 1)], tp[:])

            # ---- stats in mode space (batched over the 4 batches) ----
            Gm = cpool.tile([COUT, BLOC * KC], F32)     # [o, 66b+33ri+k]
            q4 = spool.tile([COUT, BLOC], F32, tag="q4")
            A12 = spool.tile([COUT, BLOC], F32, tag="a12")
            gmp = psS.tile([COUT, BLOC * KC], F32, tag="small")
            m1p = psS.tile([CIN, BLOC * COUT], F32, tag="small")
            for b in range(BLOC):
                nc.tensor.matmul(gmp[:, KC * b:KC * (b + 1)],
                                 ebf_s[:, 64 * b:64 * (b + 1)],
                                 Xsb[:, KC * b:KC * (b + 1)], start=True, stop=True)
                nc.tensor.matmul(m1p[:, 64 * b:64 * (b + 1)],
                                 Gsb[:, 64 * b:64 * (b + 1)],
                                 ebf_s[:, 64 * b:64 * (b + 1)], start=True, stop=True)
            nc.scalar.copy(Gm[:], gmp[:])
            em = spool.tile([CIN, BLOC * COUT], F32, tag="em")
            TT(em[:], m1p[:], ef_s[:], OP.mult)
            qp = psS.tile([COUT, BLOC], F32, tag="small")
            for b in range(BLOC):
                nc.tensor.matmul(qp[:, b:b + 1], em[:, 64 * b:64 * (b + 1)],
                                 ones_s[:], start=True, stop=True)
            nc.vector.tensor_copy(q4[:], qp[:])
            # A12 = sum_k>=1 Zr*(Zr+2Gr) + Zi*(Zi+2Gi)  (= A1 + 2*A2)
            Zall = Zsb[:].rearrange("p (ri k b) -> p b ri k", ri=2, k=M, b=4)[:, :, :, 1:M]
            Gall = Gm[:].rearrange("p (b ri k) -> p b ri k", b=BLOC, ri=2, k=M)[:, :, :, 1:M]
            w256a = spool.tile([COUT, BLOC * 64], F32, tag="w256a")
            w256b = spool.tile([COUT, BLOC * 64], F32, tag="w256b")
            wa = w256a[:].rearrange("p (b ri k) -> p b ri k", b=4, ri=2, k=M - 1)
            wb = w256b[:].rearrange("p (b ri k) -> p b ri k", b=4, ri=2, k=M - 1)
            TS(wa, Gall, 2.0, 0.0, OP.mult, OP.add)
            TT(wb, Zall, wa, OP.add)
            TT(wa, Zall, wb, OP.mult)
            for b in range(BLOC):
                nc.vector.tensor_reduce(A12[:, b:b + 1], w256a[:, 64 * b:64 * (b + 1)],
                                        mybir.AxisListType.X, OP.add)

            # vectorized S1/S2 assembly over the 4 batches
            Zr04 = Zsb[:, 0:4]                                  # Zr[k=0] per b
            u4 = Gm[:].rearrange("p (b x) -> p b x", b=BLOC, x=KC)[:, :, 0]
            v4 = spool.tile([COUT, BLOC], F32, tag="v4")
            s2c = spool.tile([COUT, BLOC], F32, tag="s2c")
            w1 = spool.tile([COUT, BLOC], F32, tag="w1")
            TT(v4[:], Zr04, u4, OP.add)
            TT(v4[:], v4[:], e4_s[:], OP.add)                   # v = Zr0+u+e
            TT(s2c[:], Zr04, Zr04, OP.mult)                     # Zr0^2
            TS(w1[:], A12[:], 2.0, 0.0, OP.mult, OP.add)
            TT(s2c[:], s2c[:], w1[:], OP.add)
            TS(w1[:], q4[:], 1.0 / L, 0.0, OP.mult, OP.add)
            TT(s2c[:], s2c[:], w1[:], OP.add)
            TT(w1[:], e4_s[:], v4[:], OP.mult)
            TS(w1[:], w1[:], 2.0, 0.0, OP.mult, OP.add)
            TT(s2c[:], s2c[:], w1[:], OP.add)
            TT(w1[:], e4_s[:], e4_s[:], OP.mult)
            TT(s2c[:], s2c[:], w1[:], OP.subtract)
            TT(w1[:], Zr04, u4, OP.mult)
            TS(w1[:], w1[:], 2.0, 0.0, OP.mult, OP.add)
            TT(s2c[:], s2c[:], w1[:], OP.add)

            stat_in = spool.tile([COUT, 2], F32, tag="stin")
            nc.vector.tensor_reduce(stat_in[:, 0:1], v4[:], mybir.AxisListType.X, OP.add)
            nc.vector.tensor_reduce(stat_in[:, 1:2], s2c[:], mybir.AxisListType.X, OP.add)

            # ---- AllReduce of (64,2) stats across the 8 cores ----
            din = dpool.tile([COUT, 2], F32)
            dout = dpool.tile([COUT, 2], F32)
            nc.gpsimd.dma_start(din[:], stat_in[:])
            if USE_AR:
                nc.gpsimd.collective_compute(
                    "AllReduce", OP.add,
                    replica_groups=[list(range(NCORES))],
                    ins=[din.opt()], outs=[dout.opt()],
                )
            else:
                nc.gpsimd.dma_start(dout[:], din[:])
            st128 = spool.tile([128, 2], F32, tag="st128")
            nc.gpsimd.dma_start(st128[0:64, :], dout[:])
            nc.gpsimd.dma_start(st128[64:128, :], dout[:])

            # replicate the ABt table into the other xtc tops via SBUF-to-SBUF
            # DMAs on the scalar HWDGE ring (off both DVE and the sync ring)
            for b in range(1, BLOC):
                nc.scalar.dma_start(xtc[b][0:64, :], xtc[0][0:64, :])

            # ---- phase C: single K=128 matmul per tile: [Z;E]^T @ [ABt;xT] ----
            OUT = []
            for j in range(2):
                outj = opool.tile([128, L], BF16, tag=f"out{j}")
                OUT.append(outj)

            # Zr0 (DC) pair-stacking via DRAM bounce (folded into normalize bias)
            zr0d = dpool.tile([COUT, BLOC], F32)
            dma(zr0d[:], Zsb[:, 0:4])
            zr0p = spool.tile([128, 2], F32, tag="zr0p")
            for j in range(2):
                dma(zr0p[0:64, j:j + 1], zr0d[:, 2 * j:2 * j + 1])
                dma(zr0p[64:128, j:j + 1], zr0d[:, 2 * j + 1:2 * j + 2])

            NSTEP = 512
            for j in range(2):
                b0, b1 = 2 * j, 2 * j + 1
                for n in range(L // NSTEP):
                    ps = psB.tile([128, NSTEP], F32, tag="invres")
                    sl = slice(NSTEP * n, NSTEP * (n + 1))
                    nc.tensor.matmul(ps[0:64, :], zeb[:, 64 * b0:64 * b0 + 64],
                                     xtc[b0][:, sl], start=True, stop=True)
                    nc.tensor.matmul(ps[64:128, :], zeb[:, 64 * b1:64 * b1 + 64],
                                     xtc[b1][:, sl], start=True, stop=True,
                                     tile_position=(0, 64))
                    nc.scalar.copy(OUT[j][:, sl], ps[:])

            # ---- BN scale/shift from all-reduced stats ----
            mean = spool.tile([128, 1], F32, tag="mean")
            ex2 = spool.tile([128, 1], F32, tag="ex2")
            var = spool.tile([128, 1], F32, tag="var")
            sv = spool.tile([128, 1], F32, tag="sv")
            sh = spool.tile([128, 1], F32, tag="sh")
            wk = spool.tile([128, 1], F32, tag="wk")
            TS(mean[:], st128[:, 0:1], 1.0 / B, 0.0, OP.mult, OP.add)
            TS(ex2[:], st128[:, 1:2], 1.0 / B, 0.0, OP.mult, OP.add)
            TT(wk[:], mean[:], mean[:], OP.mult)
            TT(var[:], ex2[:], wk[:], OP.subtract)
            TS(var[:], var[:], 1.0, EPS, OP.mult, OP.add)
            nc.scalar.activation(wk[:], var[:], mybir.ActivationFunctionType.Sqrt)
            nc.vector.reciprocal(sv[:], wk[:])
            TT(sv[:], sv[:], bnp_s[:, 0:1], OP.mult)            # s = bn_scale/std
            TT(wk[:], mean[:], sv[:], OP.mult)
            TT(sh[:], bnp_s[:, 1:2], wk[:], OP.subtract)        # shift = bias - mean*s

            bjs = []
            for j in range(2):
                bj = spool.tile([128, 1], F32, tag=f"bj{j}")
                TT(bj[:], ep_s[:, j:j + 1], zr0p[:, j:j + 1], OP.add)
                TT(bj[:], bj[:], sv[:], OP.mult)                # s*(e_b + Zr0)
                TT(bj[:], bj[:], sh[:], OP.add)                 # + shift
                bjs.append(bj)
            NQ = L // 4
            for n2 in range(4):
                for j in range(2):
                    q = slice(n2 * NQ, (n2 + 1) * NQ)
                    if (2 * n2 + j) % 2 == 0:
                        nc.scalar.activation(OUT[j][:, q], OUT[j][:, q],
                                             mybir.ActivationFunctionType.Relu,
                                             bias=bjs[j][:], scale=sv[:])
                    else:
                        TS(OUT[j][:, q], OUT[j][:, q], sv[:], bjs[j][:],
                           OP.mult, OP.add)
                        TS(OUT[j][:, q], OUT[j][:, q], 0.0, 0.0, OP.max, OP.add)
                    od = out_d[2 * j:2 * j + 2].rearrange("a b l -> (a b) l")
                    dma(od[:, q], OUT[j][:, q])

    nc.compile()
    return nc


_NC_CACHE = {}


def _get_nc():
    if "nc" not in _NC_CACHE:
        _NC_CACHE["nc"] = _build()
    return _NC_CACHE["nc"]


def _host_prep(x, t_emb, spec_w_real, spec_w_imag, dense_re, dense_im,
               conv_kernel, conv_bias, tc_weights, psi_kernel, bn_scale, bn_bias):
    """Build per-core input maps (small tensors precomputed on host)."""
    k = np.arange(M)
    l = np.arange(L)
    ang = 2.0 * np.pi * np.outer(l, k) / L
    CSt = np.concatenate([np.cos(ang) / L, -np.sin(ang) / L], axis=1)   # (L, 66)
    angk = ang[:, 1:]                                # drop DC mode
    ABt = np.concatenate([(2.0 * np.cos(angk)).T,
                          (-2.0 * np.sin(angk)).T], axis=0).astype(np.float32)

    tr = (t_emb @ dense_re).astype(np.float32)      # (B, 33)
    ti = (t_emb @ dense_im).astype(np.float32)
    psi = (t_emb @ psi_kernel).astype(np.float32)
    w_t, b_t = psi[:, :COUT], psi[:, COUT:]
    E = np.einsum("ij,bj,oj->bio", conv_kernel, w_t, tc_weights).astype(np.float32)
    e = ((conv_bias * w_t) @ tc_weights.T + b_t).astype(np.float32)      # (B, 64)

    Wcat = np.concatenate([spec_w_real, spec_w_imag], axis=2)            # (33, 64, 128)
    wm = np.ascontiguousarray(Wcat.transpose(1, 0, 2).reshape(CIN, M * 128)).astype(NP_BF16)
    cst = np.ascontiguousarray(
        CSt.reshape(NCHUNK, 128, KC).transpose(1, 0, 2).reshape(128, NCHUNK * KC)
    ).astype(NP_BF16)
    abt = ABt.astype(NP_BF16)
    idm = np.eye(64, dtype=np.float32)
    bnp = np.stack([np.tile(bn_scale, 2), np.tile(bn_bias, 2)], axis=1).astype(np.float32)

    x32 = x.astype(np.float32)
    in_maps = []
    for c in range(NCORES):
        sl = slice(BLOC * c, BLOC * (c + 1))
        xs = x32[sl]                                             # (4, L, 64)
        xb = np.ascontiguousarray(
            xs.reshape(BLOC, NCHUNK, 128, CIN).transpose(0, 2, 1, 3)
            .reshape(BLOC, 128, NCHUNK * CIN)).astype(NP_BF16)
        xt = np.ascontiguousarray(xs.transpose(0, 2, 1)).astype(NP_BF16)
        trc, tic = tr[sl], ti[sl]                                # (4, 33)
        tmod = np.concatenate([
            trc.T.reshape(-1), tic.T.reshape(-1)                 # [4k+b] each
        ]).astype(np.float32)
        tm = np.broadcast_to(tmod, (COUT, 2 * 4 * M)).copy()
        Ec = E[sl]                                               # (4, 64, 64)
        ec = e[sl]                                               # (4, 64)
        ep = np.stack([
            np.concatenate([ec[0], ec[1]]),
            np.concatenate([ec[2], ec[3]]),
        ], axis=1).astype(np.float32)                            # (128, 2)
        Ecat = np.ascontiguousarray(Ec.transpose(1, 0, 2).reshape(CIN, BLOC * COUT))
        in_maps.append({
            "xb": xb,
            "xt": xt,
            "cst": cst,
            "abt": abt,
            "wm": wm,
            "ebf": Ecat.astype(NP_BF16),
            "ef": Ecat.astype(np.float32),
            "tm": tm,
            "e4": np.ascontiguousarray(ec.T).astype(np.float32),
            "ep": ep,
            "bnp": bnp,
            "idm": idm,
        })
    return in_maps


def kernel(**inputs):
    inputs = {k: np.asarray(v) for k, v in inputs.items()}
    nc = _get_nc()
    in_maps = _host_prep(**inputs)
    res = bass_utils.run_bass_kernel_spmd(
        nc, in_maps, core_ids=list(range(NCORES)),
        trace=bool(int(os.environ.get("KBENCH_TRACE", "0"))),
    )
    out = np.empty((B, L, COUT), np.float32)
    for c in range(NCORES):
        o = res.results[c]["out"].astype(np.float32)     # (4, 64, L)
        out[BLOC * c:BLOC * (c + 1)] = np.ascontiguousarray(o.transpose(0, 2, 1))
    _NC_CACHE["last_results"] = res
    return out



# revision 3
# speedup vs baseline: 1.2430x; 1.2430x over previous
"""Trainium2 Bass kernel for CTUNOBlock1D (spectral conv + time conv + batchnorm + relu).

Strategy (data-parallel over batch, 8 cores, 4 batches/core):
  - rfft/irfft use only 33 modes -> implemented as DFT matmuls against small
    trig tables (bf16, f32 accumulation on PE).
  - phase A fuses the forward DFT and the Gram matrix x^T x into ONE matmul
    per (chunk, batch): the host prepends the 66 DFT-table columns to each
    128-row x chunk, so a single stationary load of the x chunk streams
    [cst_u | x_u] and accumulates [X | G] in one PSUM tile.
  - BN statistics are computed exactly in mode space (Parseval) right after
    phase A, so the tiny (64,2) stats AllReduce overlaps the entire
    inverse/residual phase (gpsimd ring carries only the AR traffic).
  - residual branch is folded on host: E_b = K diag(w_t) Wt^T, e_b bias; the
    device computes out^T = Z^T @ ABt + E_b^T @ x^T per batch via one K=128
    matmul per tile against [ABt; x^T] tiles (ABt baked in on host), with
    PSUM->SBUF copies alternating between scalar and vector engines.
  - BN scale/shift (+ folded e_b) + ReLU applied in 16 units alternating
    scalar/vector, each unit's output DMA issued from that engine's own ring.
  - output is written channel-major and transposed on host.
"""

import os
import numpy as np

import concourse.bass as bass
import concourse.mybir as mybir
import concourse.bacc as bacc
import concourse.tile as tile
from concourse import bass_utils

F32 = mybir.dt.float32
BF16 = mybir.dt.bfloat16
NP_BF16 = mybir.dt.np(BF16)

B, L, CIN, COUT, TEMB = 32, 8192, 64, 64, 256
M = 33            # retained rfft modes
KC = 2 * M        # 66 (real|imag concat)
W = KC + CIN      # 130 fused phase-A block width
NCORES = 8
BLOC = B // NCORES   # 4 batches per core
EPS = 1e-5
NCHUNK = L // 128    # 64 l-chunks of 128
USE_AR = bool(int(os.environ.get("KBENCH_AR", "1")))


def _build():
    nc = bacc.Bacc(None, target_bir_lowering=False)

    xb_d = nc.dram_tensor("xb", [BLOC, 128, NCHUNK * W], BF16, kind="ExternalInput")
    xt_d = nc.dram_tensor("xt", [BLOC, 128, L], BF16, kind="ExternalInput")
    wm_d = nc.dram_tensor("wm", [CIN, M * 128], BF16, kind="ExternalInput")
    ebf_d = nc.dram_tensor("ebf", [CIN, BLOC * COUT], BF16, kind="ExternalInput")
    ef_d = nc.dram_tensor("ef", [CIN, BLOC * COUT], F32, kind="ExternalInput")
    tm_d = nc.dram_tensor("tm", [COUT, 2 * 4 * M], F32, kind="ExternalInput")
    e4_d = nc.dram_tensor("e4", [COUT, BLOC], F32, kind="ExternalInput")
    ep_d = nc.dram_tensor("ep", [128, 2], F32, kind="ExternalInput")
    bnp_d = nc.dram_tensor("bnp", [128, 2], F32, kind="ExternalInput")
    id_d = nc.dram_tensor("idm", [64, 64], F32, kind="ExternalInput")
    out_d = nc.dram_tensor("out", [BLOC, COUT, L], BF16, kind="ExternalOutput")

    with tile.TileContext(nc) as tc:
        with (
            tc.tile_pool(name="const", bufs=1) as cpool,
            tc.tile_pool(name="xs", bufs=1) as xpool,
            tc.tile_pool(name="xtp", bufs=1) as xtpool,
            tc.tile_pool(name="outb", bufs=1) as opool,
            tc.tile_pool(name="small", bufs=2) as spool,
            tc.tile_pool(name="psA", bufs=2, space=bass.MemorySpace.PSUM) as psA,
            tc.tile_pool(name="psS", bufs=2, space=bass.MemorySpace.PSUM) as psS,
            tc.tile_pool(name="psB", bufs=2, space=bass.MemorySpace.PSUM) as psB,
            tc.tile_pool(name="dram", bufs=1, space=bass.MemorySpace.DRAM) as dpool,
        ):
            dma = nc.sync.dma_start

            # ---- phase-A critical loads first, then phase-C bulk ----
            xss = []
            for b in range(BLOC):
                xs = xpool.tile([128, NCHUNK * W], BF16, tag=f"xs{b}")
                dma(xs[:], xb_d[b])
                xss.append(xs)

            # small constants (phase B/stats path)
            wm_s = cpool.tile([CIN, M * 128], BF16)
            tm_s = cpool.tile([COUT, 2 * 4 * M], F32)
            e4_s = cpool.tile([COUT, BLOC], F32)
            ep_s = cpool.tile([128, 2], F32)
            bnp_s = cpool.tile([128, 2], F32)
            id_s = cpool.tile([64, 64], F32)
            ones_s = cpool.tile([64, 1], F32)
            dma(wm_s[:], wm_d[:])
            dma(tm_s[:], tm_d[:])
            dma(e4_s[:], e4_d[:])
            dma(ep_s[:], ep_d[:])
            dma(bnp_s[:], bnp_d[:])
            dma(id_s[:], id_d[:])
            nc.vector.memset(ones_s[:], 1.0)

            ebf_s = cpool.tile([CIN, BLOC * COUT], BF16)   # [i, 64b+o]
            ef_s = cpool.tile([CIN, BLOC * COUT], F32)
            zeb = cpool.tile([128, BLOC * COUT], BF16)     # [0:64]=Z^T, [64:128]=E
            dma(ebf_s[:], ebf_d[:])
            dma(ef_s[:], ef_d[:])
            dma(zeb[64:128, :], ebf_d[:])

            # phase-C bulk: [ABt; x^T] per batch, ABt baked in on host
            xtc = []
            for b in range(BLOC):
                xt = xtpool.tile([128, L], BF16, tag=f"xtc{b}")
                dma(xt[:], xt_d[b])
                xtc.append(xt)

            # early dummy Sqrt to pre-load the ACT table set
            warm = spool.tile([1, 1], F32)
            nc.vector.memset(warm[:], 1.0)
            nc.scalar.activation(warm[:], warm[:], mybir.ActivationFunctionType.Sqrt)

            # ---- phase A: fused forward DFT + Gram per batch ----
            Xsb = cpool.tile([CIN, BLOC * KC], BF16)    # [c, 66b+kcat]
            Gsb = cpool.tile([CIN, BLOC * CIN], BF16)   # [c, 64b+c']
            for b in range(BLOC):
                xs = xss[b]
                xg = psA.tile([CIN, W], F32, tag="xg")
                for u in range(NCHUNK):
                    nc.tensor.matmul(xg[:], xs[:, W * u + KC:W * (u + 1)],
                                     xs[:, W * u:W * (u + 1)],
                                     start=(u == 0), stop=(u == NCHUNK - 1))
                nc.vector.tensor_copy(Xsb[:, KC * b:KC * (b + 1)], xg[:, 0:KC])
                nc.vector.tensor_copy(Gsb[:, CIN * b:CIN * (b + 1)], xg[:, KC:W])

            # ---- phase B: mode mixing ----
            # P12[0:64, 8k+x] = Wr[k]^T @ X_x ; P12[64:128] likewise for Wi
            # (x = 4j+b, j=0: Xr, 1: Xi)
            P12 = psS.tile([128, M * 8], F32, tag="p12")
            Xr4 = Xsb[:].rearrange("p (b j k) -> p j b k", b=BLOC, j=2, k=M)
            for k in range(M):
                nc.tensor.matmul(P12[:, 8 * k:8 * (k + 1)],
                                 wm_s[:, 128 * k:128 * (k + 1)],
                                 Xr4[:, :, :, k], start=True, stop=True)

            TT = nc.vector.tensor_tensor
            TS = nc.vector.tensor_scalar
            OP = mybir.AluOpType
            Psb = spool.tile([COUT, 2 * M * 8], F32, tag="psb")
            nc.scalar.copy(Psb[:, 0:M * 8], P12[0:64, :])
            nc.scalar.copy(Psb[:, M * 8:2 * M * 8], P12[64:128, :])
            Yr = spool.tile([COUT, 4 * M], F32, tag="yr")   # [(k,b)] = 4k+b
            Yi = spool.tile([COUT, 4 * M], F32, tag="yi")
            Pk1 = Psb[:, 0:M * 8].rearrange("p (k x) -> p k x", k=M, x=8)
            Pk2 = Psb[:, M * 8:2 * M * 8].rearrange("p (k x) -> p k x", k=M, x=8)
            Yrv = Yr[:].rearrange("p (k b) -> p k b", k=M, b=4)
            Yiv = Yi[:].rearrange("p (k b) -> p k b", k=M, b=4)
            TT(Yrv, Pk1[:, :, 0:4], Pk2[:, :, 4:8], OP.subtract)
            TT(Yiv, Pk2[:, :, 0:4], Pk1[:, :, 4:8], OP.add)

            Zsb = cpool.tile([COUT, 2 * 4 * M], F32)  # [(ri,k,b)] = 132ri+4k+b
            t1 = spool.tile([COUT, 4 * M], F32, tag="t1")
            t2 = spool.tile([COUT, 4 * M], F32, tag="t2")
            ntm = 4 * M
            TT(t1[:], Yr[:], tm_s[:, 0:ntm], OP.mult)
            TT(t2[:], Yi[:], tm_s[:, ntm:2 * ntm], OP.mult)
            TT(Zsb[:, 0:ntm], t1[:], t2[:], OP.subtract)
            TT(t1[:], Yi[:], tm_s[:, 0:ntm], OP.mult)
            TT(t2[:], Yr[:], tm_s[:, ntm:2 * ntm], OP.mult)
            TT(Zsb[:, ntm:2 * ntm], t1[:], t2[:], OP.add)

            # ---- Z transpose (per batch, DC row folded into bias later) ----
            # zeb[b]: rows 0:64 = Z^T (modes k=1..32, re|im), rows 64:128 = E_b
            Zflat = spool.tile([COUT, 4 * 64], F32, tag="zflat")  # [b][ri,k>=1]
            nc.vector.tensor_copy(
                Zflat[:].rearrange("p (b ri k) -> p b ri k", b=4, ri=2, k=M - 1),
                Zsb[:].rearrange("p (ri k b) -> p b ri k", ri=2, k=M, b=4)[:, :, :, 1:M])
            for b in range(BLOC):
                tp = psS.tile([CIN, COUT], F32, tag="small")
                nc.tensor.transpose(tp[:], Zflat[:, 64 * b:64 * (b + 1)], id_s[:])
                nc.vector.tensor_copy(zeb[0:64, 64 * b:64 * (b + 1)], tp[:])

            # ---- stats in mode space (batched over the 4 batches) ----
            Gm = cpool.tile([COUT, BLOC * KC], F32)     # [o, 66b+33ri+k]
            q4 = spool.tile([COUT, BLOC], F32, tag="q4")
            A12 = spool.tile([COUT, BLOC], F32, tag="a12")
            gmp = psS.tile([COUT, BLOC * KC], F32, tag="small")
            m1p = psS.tile([CIN, BLOC * COUT], F32, tag="small")
            for b in range(BLOC):
                nc.tensor.matmul(gmp[:, KC * b:KC * (b + 1)],
                                 ebf_s[:, 64 * b:64 * (b + 1)],
                                 Xsb[:, KC * b:KC * (b + 1)], start=True, stop=True)
                nc.tensor.matmul(m1p[:, 64 * b:64 * (b + 1)],
                                 Gsb[:, 64 * b:64 * (b + 1)],
                                 ebf_s[:, 64 * b:64 * (b + 1)], start=True, stop=True)
            nc.scalar.copy(Gm[:], gmp[:])
            em = spool.tile([CIN, BLOC * COUT], F32, tag="em")
            TT(em[:], m1p[:], ef_s[:], OP.mult)
            qp = psS.tile([COUT, BLOC], F32, tag="small")
            for b in range(BLOC):
                nc.tensor.matmul(qp[:, b:b + 1], em[:, 64 * b:64 * (b + 1)],
                                 ones_s[:], start=True, stop=True)
            nc.vector.tensor_copy(q4[:], qp[:])
            # A12 = sum_k>=1 Zr*(Zr+2Gr) + Zi*(Zi+2Gi)  (= A1 + 2*A2)
            Zall = Zsb[:].rearrange("p (ri k b) -> p b ri k", ri=2, k=M, b=4)[:, :, :, 1:M]
            Gall = Gm[:].rearrange("p (b ri k) -> p b ri k", b=BLOC, ri=2, k=M)[:, :, :, 1:M]
            w256a = spool.tile([COUT, BLOC * 64], F32, tag="w256a")
            w256b = spool.tile([COUT, BLOC * 64], F32, tag="w256b")
            wa = w256a[:].rearrange("p (b ri k) -> p b ri k", b=4, ri=2, k=M - 1)
            wb = w256b[:].rearrange("p (b ri k) -> p b ri k", b=4, ri=2, k=M - 1)
            TS(wa, Gall, 2.0, 0.0, OP.mult, OP.add)
            TT(wb, Zall, wa, OP.add)
            TT(wa, Zall, wb, OP.mult)
            for b in range(BLOC):
                nc.vector.tensor_reduce(A12[:, b:b + 1], w256a[:, 64 * b:64 * (b + 1)],
                                        mybir.AxisListType.X, OP.add)

            # vectorized S1/S2 assembly over the 4 batches
            Zr04 = Zsb[:, 0:4]                                  # Zr[k=0] per b
            u4 = Gm[:].rearrange("p (b x) -> p b x", b=BLOC, x=KC)[:, :, 0]
            v4 = spool.tile([COUT, BLOC], F32, tag="v4")
            s2c = spool.tile([COUT, BLOC], F32, tag="s2c")
            w1 = spool.tile([COUT, BLOC], F32, tag="w1")
            TT(v4[:], Zr04, u4, OP.add)
            TT(v4[:], v4[:], e4_s[:], OP.add)                   # v = Zr0+u+e
            TT(s2c[:], Zr04, Zr04, OP.mult)                     # Zr0^2
            TS(w1[:], A12[:], 2.0, 0.0, OP.mult, OP.add)
            TT(s2c[:], s2c[:], w1[:], OP.add)
            TS(w1[:], q4[:], 1.0 / L, 0.0, OP.mult, OP.add)
            TT(s2c[:], s2c[:], w1[:], OP.add)
            TT(w1[:], e4_s[:], v4[:], OP.mult)
            TS(w1[:], w1[:], 2.0, 0.0, OP.mult, OP.add)
            TT(s2c[:], s2c[:], w1[:], OP.add)
            TT(w1[:], e4_s[:], e4_s[:], OP.mult)
            TT(s2c[:], s2c[:], w1[:], OP.subtract)
            TT(w1[:], Zr04, u4, OP.mult)
            TS(w1[:], w1[:], 2.0, 0.0, OP.mult, OP.add)
            TT(s2c[:], s2c[:], w1[:], OP.add)

            stat_in = spool.tile([COUT, 2], F32, tag="stin")
            nc.vector.tensor_reduce(stat_in[:, 0:1], v4[:], mybir.AxisListType.X, OP.add)
            nc.vector.tensor_reduce(stat_in[:, 1:2], s2c[:], mybir.AxisListType.X, OP.add)

            # ---- AllReduce of (64,2) stats across the 8 cores ----
            # (gpsimd ring carries ONLY this traffic, so it fires immediately)
            din = dpool.tile([COUT, 2], F32)
            dout = dpool.tile([COUT, 2], F32)
            nc.gpsimd.dma_start(din[:], stat_in[:])
            if USE_AR:
                nc.gpsimd.collective_compute(
                    "AllReduce", OP.add,
                    replica_groups=[list(range(NCORES))],
                    ins=[din.opt()], outs=[dout.opt()],
                )
            else:
                nc.gpsimd.dma_start(dout[:], din[:])
            st128 = spool.tile([128, 2], F32, tag="st128")
            nc.gpsimd.dma_start(st128[0:64, :], dout[:])
            nc.gpsimd.dma_start(st128[64:128, :], dout[:])

            # ---- phase C: single K=128 matmul per tile: [Z;E]^T @ [ABt;xT] ----
            OUT = []
            for j in range(2):
                outj = opool.tile([128, L], BF16, tag=f"out{j}")
                OUT.append(outj)

            # Zr0 (DC) pair-stacking via DRAM bounce (folded into normalize bias)
            zr0d = dpool.tile([COUT, BLOC], F32)
            dma(zr0d[:], Zsb[:, 0:4])
            zr0p = spool.tile([128, 2], F32, tag="zr0p")
            for j in range(2):
                dma(zr0p[0:64, j:j + 1], zr0d[:, 2 * j:2 * j + 1])
                dma(zr0p[64:128, j:j + 1], zr0d[:, 2 * j + 1:2 * j + 2])

            NSTEP = 512
            for j in range(2):
                b0, b1 = 2 * j, 2 * j + 1
                for n in range(L // NSTEP):
                    ps = psB.tile([128, NSTEP], F32, tag="invres")
                    sl = slice(NSTEP * n, NSTEP * (n + 1))
                    nc.tensor.matmul(ps[0:64, :], zeb[:, 64 * b0:64 * b0 + 64],
                                     xtc[b0][:, sl], start=True, stop=True)
                    nc.tensor.matmul(ps[64:128, :], zeb[:, 64 * b1:64 * b1 + 64],
                                     xtc[b1][:, sl], start=True, stop=True,
                                     tile_position=(0, 64))
                    if n % 2 == 0:
                        nc.scalar.copy(OUT[j][:, sl], ps[:])
                    else:
                        nc.vector.tensor_copy(OUT[j][:, sl], ps[:])

            # ---- BN scale/shift from all-reduced stats ----
            mean = spool.tile([128, 1], F32, tag="mean")
            ex2 = spool.tile([128, 1], F32, tag="ex2")
            var = spool.tile([128, 1], F32, tag="var")
            sv = spool.tile([128, 1], F32, tag="sv")
            sh = spool.tile([128, 1], F32, tag="sh")
            wk = spool.tile([128, 1], F32, tag="wk")
            TS(mean[:], st128[:, 0:1], 1.0 / B, 0.0, OP.mult, OP.add)
            TS(ex2[:], st128[:, 1:2], 1.0 / B, 0.0, OP.mult, OP.add)
            TT(wk[:], mean[:], mean[:], OP.mult)
            TT(var[:], ex2[:], wk[:], OP.subtract)
            TS(var[:], var[:], 1.0, EPS, OP.mult, OP.add)
            nc.scalar.activation(wk[:], var[:], mybir.ActivationFunctionType.Sqrt)
            nc.vector.reciprocal(sv[:], wk[:])
            TT(sv[:], sv[:], bnp_s[:, 0:1], OP.mult)            # s = bn_scale/std
            TT(wk[:], mean[:], sv[:], OP.mult)
            TT(sh[:], bnp_s[:, 1:2], wk[:], OP.subtract)        # shift = bias - mean*s

            bjs = []
            for j in range(2):
                bj = spool.tile([128, 1], F32, tag=f"bj{j}")
                TT(bj[:], ep_s[:, j:j + 1], zr0p[:, j:j + 1], OP.add)
                TT(bj[:], bj[:], sv[:], OP.mult)                # s*(e_b + Zr0)
                TT(bj[:], bj[:], sh[:], OP.add)                 # + shift
                bjs.append(bj)
            NQ = L // 8
            for n2 in range(8):
                for j in range(2):
                    q = slice(n2 * NQ, (n2 + 1) * NQ)
                    od = out_d[2 * j:2 * j + 2].rearrange("a b l -> (a b) l")
                    if (2 * n2 + j) % 2 == 0:
                        nc.scalar.activation(OUT[j][:, q], OUT[j][:, q],
                                             mybir.ActivationFunctionType.Relu,
                                             bias=bjs[j][:], scale=sv[:])
                        nc.scalar.dma_start(od[:, q], OUT[j][:, q])
                    else:
                        TS(OUT[j][:, q], OUT[j][:, q], sv[:], bjs[j][:],
                           OP.mult, OP.add)
                        TS(OUT[j][:, q], OUT[j][:, q], 0.0, 0.0, OP.max, OP.add)
                        dma(od[:, q], OUT[j][:, q])

    nc.compile()
    return nc


_NC_CACHE = {}


def _get_nc():
    if "nc" not in _NC_CACHE:
        _NC_CACHE["nc"] = _build()
    return _NC_CACHE["nc"]


def _host_prep(x, t_emb, spec_w_real, spec_w_imag, dense_re, dense_im,
               conv_kernel, conv_bias, tc_weights, psi_kernel, bn_scale, bn_bias):
    """Build per-core input maps (small tensors precomputed on host)."""
    k = np.arange(M)
    l = np.arange(L)
    ang = 2.0 * np.pi * np.outer(l, k) / L
    CSt = np.concatenate([np.cos(ang) / L, -np.sin(ang) / L], axis=1)   # (L, 66)
    angk = ang[:, 1:]                                # drop DC mode
    ABt = np.concatenate([(2.0 * np.cos(angk)).T,
                          (-2.0 * np.sin(angk)).T], axis=0).astype(np.float32)

    tr = (t_emb @ dense_re).astype(np.float32)      # (B, 33)
    ti = (t_emb @ dense_im).astype(np.float32)
    psi = (t_emb @ psi_kernel).astype(np.float32)
    w_t, b_t = psi[:, :COUT], psi[:, COUT:]
    E = np.einsum("ij,bj,oj->bio", conv_kernel, w_t, tc_weights).astype(np.float32)
    e = ((conv_bias * w_t) @ tc_weights.T + b_t).astype(np.float32)      # (B, 64)

    Wcat = np.concatenate([spec_w_real, spec_w_imag], axis=2)            # (33, 64, 128)
    wm = np.ascontiguousarray(Wcat.transpose(1, 0, 2).reshape(CIN, M * 128)).astype(NP_BF16)
    # cst chunks: (NCHUNK, 128, 66)
    cstc = CSt.reshape(NCHUNK, 128, KC).astype(NP_BF16)
    abt16 = ABt.astype(NP_BF16)                      # (64, L)
    idm = np.eye(64, dtype=np.float32)
    bnp = np.stack([np.tile(bn_scale, 2), np.tile(bn_bias, 2)], axis=1).astype(np.float32)

    x32 = x.astype(np.float32)
    in_maps = []
    for c in range(NCORES):
        sl = slice(BLOC * c, BLOC * (c + 1))
        xs = x32[sl]                                             # (4, L, 64)
        # fused phase-A layout: per chunk u, block = [cst_u | x_u]
        xb = np.empty((BLOC, NCHUNK, 128, W), dtype=NP_BF16)
        xck = xs.reshape(BLOC, NCHUNK, 128, CIN).astype(NP_BF16)
        xb[:, :, :, :KC] = cstc[None]
        xb[:, :, :, KC:] = xck
        xb = np.ascontiguousarray(
            xb.transpose(0, 2, 1, 3).reshape(BLOC, 128, NCHUNK * W))
        # phase-C layout: [ABt; x^T] per batch
        xt = np.empty((BLOC, 128, L), dtype=NP_BF16)
        xt[:, 0:64, :] = abt16[None]
        xt[:, 64:128, :] = xs.transpose(0, 2, 1).astype(NP_BF16)
        trc, tic = tr[sl], ti[sl]                                # (4, 33)
        tmod = np.concatenate([
            trc.T.reshape(-1), tic.T.reshape(-1)                 # [4k+b] each
        ]).astype(np.float32)
        tm = np.broadcast_to(tmod, (COUT, 2 * 4 * M)).copy()
        Ec = E[sl]                                               # (4, 64, 64)
        ec = e[sl]                                               # (4, 64)
        ep = np.stack([
            np.concatenate([ec[0], ec[1]]),
            np.concatenate([ec[2], ec[3]]),
        ], axis=1).astype(np.float32)                            # (128, 2)
        Ecat = np.ascontiguousarray(Ec.transpose(1, 0, 2).reshape(CIN, BLOC * COUT))
        in_maps.append({
            "xb": xb,
            "xt": xt,
            "wm": wm,
            "ebf": Ecat.astype(NP_BF16),
            "ef": Ecat.astype(np.float32),
            "tm": tm,
            "e4": np.ascontiguousarray(ec.T).astype(np.float32),
            "ep": ep,
            "bnp": bnp,
            "idm": idm,
        })
    return in_maps


def kernel(**inputs):
    inputs = {k: np.asarray(v) for k, v in inputs.items()}
    nc = _get_nc()
    in_maps = _host_prep(**inputs)
    res = bass_utils.run_bass_kernel_spmd(
        nc, in_maps, core_ids=list(range(NCORES)),
        trace=bool(int(os.environ.get("KBENCH_TRACE", "0"))),
    )
    out = np.empty((B, L, COUT), np.float32)
    for c in range(NCORES):
        o = res.results[c]["out"].astype(np.float32)     # (4, 64, L)
        out[BLOC * c:BLOC * (c + 1)] = np.ascontiguousarray(o.transpose(0, 2, 1))
    _NC_CACHE["last_results"] = res
    return out


# revision 7
# speedup vs baseline: 1.2753x; 1.0260x over previous
"""Trainium2 Bass kernel for CTUNOBlock1D (spectral conv + time conv + batchnorm + relu).

Strategy (data-parallel over batch, 8 cores, 4 batches/core):
  - rfft/irfft use only 33 modes -> implemented as DFT matmuls against small
    trig tables (bf16, f32 accumulation on PE).
  - phase A computes the forward DFT and the Gram matrix x^T x per batch,
    sharing each x chunk as the stationary operand of two back-to-back
    matmuls (DFT table rhs, then the chunk itself).
  - BN statistics are computed exactly in mode space (Parseval) right after
    the mode modulation, so the tiny (64,2) stats AllReduce is triggered as
    early as possible and overlaps the inverse/residual phase (the gpsimd
    ring carries only the AR traffic).
  - residual branch is folded on host: E_b = K diag(w_t) Wt^T, e_b bias; the
    device computes out^T = Z^T @ ABt + E_b^T @ x^T per batch via one K=128
    matmul per tile against [ABt; x^T] tiles (ABt replicated SBUF->SBUF on
    the scalar ring), with PSUM->SBUF copies alternating scalar/vector.
  - BN scale/shift (+ folded e_b) + ReLU applied in 16 units split between
    the vector (10) and scalar (6) engines, each unit's output DMA issued
    from an engine-local ring.
  - output is written channel-major and transposed on host.
"""

import os
import numpy as np

import concourse.bass as bass
import concourse.mybir as mybir
import concourse.bacc as bacc
import concourse.tile as tile
from concourse import bass_utils

F32 = mybir.dt.float32
BF16 = mybir.dt.bfloat16
NP_BF16 = mybir.dt.np(BF16)

B, L, CIN, COUT, TEMB = 32, 8192, 64, 64, 256
M = 33            # retained rfft modes
KC = 2 * M        # 66 (real|imag concat)
NCORES = 8
BLOC = B // NCORES   # 4 batches per core
EPS = 1e-5
NCHUNK = L // 128    # 64 l-chunks of 128
USE_AR = bool(int(os.environ.get("KBENCH_AR", "1")))


def _build():
    nc = bacc.Bacc(None, target_bir_lowering=False)

    xb_d = nc.dram_tensor("xb", [BLOC, 128, NCHUNK * CIN], BF16, kind="ExternalInput")
    cst_d = nc.dram_tensor("cst", [128, NCHUNK * KC], BF16, kind="ExternalInput")
    xt_d = nc.dram_tensor("xt", [BLOC, CIN, L], BF16, kind="ExternalInput")
    abt_d = nc.dram_tensor("abt", [CIN, L], BF16, kind="ExternalInput")
    wm_d = nc.dram_tensor("wm", [CIN, M * 128], BF16, kind="ExternalInput")
    ebf_d = nc.dram_tensor("ebf", [CIN, BLOC * COUT], BF16, kind="ExternalInput")
    ef_d = nc.dram_tensor("ef", [CIN, BLOC * COUT], F32, kind="ExternalInput")
    tm_d = nc.dram_tensor("tm", [COUT, 2 * 4 * M], F32, kind="ExternalInput")
    e4_d = nc.dram_tensor("e4", [COUT, BLOC], F32, kind="ExternalInput")
    ep_d = nc.dram_tensor("ep", [128, 2], F32, kind="ExternalInput")
    bnp_d = nc.dram_tensor("bnp", [128, 2], F32, kind="ExternalInput")
    id_d = nc.dram_tensor("idm", [64, 64], F32, kind="ExternalInput")
    out_d = nc.dram_tensor("out", [BLOC, COUT, L], BF16, kind="ExternalOutput")

    with tile.TileContext(nc) as tc:
        with (
            tc.tile_pool(name="const", bufs=1) as cpool,
            tc.tile_pool(name="xs", bufs=1) as xpool,
            tc.tile_pool(name="xtp", bufs=1) as xtpool,
            tc.tile_pool(name="outb", bufs=1) as opool,
            tc.tile_pool(name="small", bufs=2) as spool,
            tc.tile_pool(name="psA", bufs=1, space=bass.MemorySpace.PSUM) as psA,
            tc.tile_pool(name="psS", bufs=2, space=bass.MemorySpace.PSUM) as psS,
            tc.tile_pool(name="psB", bufs=2, space=bass.MemorySpace.PSUM) as psB,
            tc.tile_pool(name="dram", bufs=1, space=bass.MemorySpace.DRAM) as dpool,
        ):
            dma = nc.sync.dma_start

            # ---- phase-A critical loads first, then phase-C bulk ----
            cst_s = cpool.tile([128, NCHUNK * KC], BF16)
            dma(cst_s[:], cst_d[:])
            xss = []
            for b in range(BLOC):
                xs = xpool.tile([128, NCHUNK * CIN], BF16, tag=f"xs{b}")
                dma(xs[:], xb_d[b])
                xss.append(xs)

            # small constants (phase B/stats path)
            wm_s = cpool.tile([CIN, M * 128], BF16)
            tm_s = cpool.tile([COUT, 2 * 4 * M], F32)
            e4_s = cpool.tile([COUT, BLOC], F32)
            ep_s = cpool.tile([128, 2], F32)
            bnp_s = cpool.tile([128, 2], F32)
            id_s = cpool.tile([64, 64], F32)
            ones_s = cpool.tile([64, 1], F32)
            dma(wm_s[:], wm_d[:])
            dma(tm_s[:], tm_d[:])
            dma(e4_s[:], e4_d[:])
            dma(ep_s[:], ep_d[:])
            dma(bnp_s[:], bnp_d[:])
            dma(id_s[:], id_d[:])
            nc.vector.memset(ones_s[:], 1.0)

            ebf_s = cpool.tile([CIN, BLOC * COUT], BF16)   # [i, 64b+o]
            ef_s = cpool.tile([CIN, BLOC * COUT], F32)
            zeb = cpool.tile([128, BLOC * COUT], BF16)     # [0:64]=Z^T, [64:128]=E
            dma(ebf_s[:], ebf_d[:])
            dma(ef_s[:], ef_d[:])
            dma(zeb[64:128, :], ebf_d[:])

            # phase-C bulk: [ABt; x^T] per batch; ABt lands in xtc[0] and is
            # replicated SBUF->SBUF into the other tops on the scalar ring
            xtc = []
            for b in range(BLOC):
                xt = xtpool.tile([128, L], BF16, tag=f"xtc{b}")
                xtc.append(xt)
            dma(xtc[0][0:64, :], abt_d[:])
            for b in range(BLOC):
                dma(xtc[b][64:128, :], xt_d[b])
            for b in range(1, BLOC):
                nc.scalar.dma_start(xtc[b][0:64, :], xtc[0][0:64, :])

            # early dummy Sqrt to pre-load the ACT table set
            warm = spool.tile([1, 1], F32)
            nc.vector.memset(warm[:], 1.0)
            nc.scalar.activation(warm[:], warm[:], mybir.ActivationFunctionType.Sqrt)

            # ---- phase A: forward DFT + Gram per batch ----
            Xsb = cpool.tile([CIN, BLOC * KC], BF16)    # [c, 66b+kcat]
            Gsb = cpool.tile([CIN, BLOC * CIN], BF16)   # [c, 64b+c']
            for b in range(BLOC):
                xs = xss[b]
                xacc = psA.tile([CIN, KC], F32, tag="xacc")
                gacc = psA.tile([CIN, CIN], F32, tag="gacc")
                for u in range(NCHUNK):
                    lhs = xs[:, CIN * u:CIN * (u + 1)]
                    nc.tensor.matmul(xacc[:], lhs, cst_s[:, KC * u:KC * (u + 1)],
                                     start=(u == 0), stop=(u == NCHUNK - 1))
                    nc.tensor.matmul(gacc[:], lhs, lhs,
                                     start=(u == 0), stop=(u == NCHUNK - 1))
                nc.vector.tensor_copy(Xsb[:, KC * b:KC * (b + 1)], xacc[:])
                nc.vector.tensor_copy(Gsb[:, CIN * b:CIN * (b + 1)], gacc[:])

            # ---- phase B: mode mixing ----
            # P12[0:64, 8k+x] = Wr[k]^T @ X_x ; P12[64:128] likewise for Wi
            # (x = 4j+b, j=0: Xr, 1: Xi)
            P12 = psS.tile([128, M * 8], F32, tag="p12")
            Xr4 = Xsb[:].rearrange("p (b j k) -> p j b k", b=BLOC, j=2, k=M)
            for k in range(M):
                nc.tensor.matmul(P12[:, 8 * k:8 * (k + 1)],
                                 wm_s[:, 128 * k:128 * (k + 1)],
                                 Xr4[:, :, :, k], start=True, stop=True)

            TT = nc.vector.tensor_tensor
            TS = nc.vector.tensor_scalar
            OP = mybir.AluOpType
            Psb = spool.tile([COUT, 2 * M * 8], F32, tag="psb")
            nc.scalar.copy(Psb[:, 0:M * 8], P12[0:64, :])
            nc.scalar.copy(Psb[:, M * 8:2 * M * 8], P12[64:128, :])
            Yr = spool.tile([COUT, 4 * M], F32, tag="yr")   # [(k,b)] = 4k+b
            Yi = spool.tile([COUT, 4 * M], F32, tag="yi")
            Pk1 = Psb[:, 0:M * 8].rearrange("p (k x) -> p k x", k=M, x=8)
            Pk2 = Psb[:, M * 8:2 * M * 8].rearrange("p (k x) -> p k x", k=M, x=8)
            Yrv = Yr[:].rearrange("p (k b) -> p k b", k=M, b=4)
            Yiv = Yi[:].rearrange("p (k b) -> p k b", k=M, b=4)
            TT(Yrv, Pk1[:, :, 0:4], Pk2[:, :, 4:8], OP.subtract)
            TT(Yiv, Pk2[:, :, 0:4], Pk1[:, :, 4:8], OP.add)

            Zsb = cpool.tile([COUT, 2 * 4 * M], F32)  # [(ri,k,b)] = 132ri+4k+b
            t1 = spool.tile([COUT, 4 * M], F32, tag="t1")
            t2 = spool.tile([COUT, 4 * M], F32, tag="t2")
            ntm = 4 * M
            TT(t1[:], Yr[:], tm_s[:, 0:ntm], OP.mult)
            TT(t2[:], Yi[:], tm_s[:, ntm:2 * ntm], OP.mult)
            TT(Zsb[:, 0:ntm], t1[:], t2[:], OP.subtract)
            TT(t1[:], Yi[:], tm_s[:, 0:ntm], OP.mult)
            TT(t2[:], Yr[:], tm_s[:, ntm:2 * ntm], OP.mult)
            TT(Zsb[:, ntm:2 * ntm], t1[:], t2[:], OP.add)

            # ---- stats in mode space, FIRST (AR trigger asap) ----
            Gm = cpool.tile([COUT, BLOC * KC], F32)     # [o, 66b+33ri+k]
            q4 = spool.tile([COUT, BLOC], F32, tag="q4")
            A12 = spool.tile([COUT, BLOC], F32, tag="a12")
            gmp = psS.tile([COUT, BLOC * KC], F32, tag="small")
            m1p = psS.tile([CIN, BLOC * COUT], F32, tag="small")
            for b in range(BLOC):
                nc.tensor.matmul(gmp[:, KC * b:KC * (b + 1)],
                                 ebf_s[:, 64 * b:64 * (b + 1)],
                                 Xsb[:, KC * b:KC * (b + 1)], start=True, stop=True)
                nc.tensor.matmul(m1p[:, 64 * b:64 * (b + 1)],
                                 Gsb[:, 64 * b:64 * (b + 1)],
                                 ebf_s[:, 64 * b:64 * (b + 1)], start=True, stop=True)
            nc.scalar.copy(Gm[:], gmp[:])
            em = spool.tile([CIN, BLOC * COUT], F32, tag="em")
            TT(em[:], m1p[:], ef_s[:], OP.mult)
            qp = psS.tile([COUT, BLOC], F32, tag="small")
            for b in range(BLOC):
                nc.tensor.matmul(qp[:, b:b + 1], em[:, 64 * b:64 * (b + 1)],
                                 ones_s[:], start=True, stop=True)
            nc.vector.tensor_copy(q4[:], qp[:])
            # A12 = sum_k>=1 Zr*(Zr+2Gr) + Zi*(Zi+2Gi)  (= A1 + 2*A2)
            Zall = Zsb[:].rearrange("p (ri k b) -> p b ri k", ri=2, k=M, b=4)[:, :, :, 1:M]
            Gall = Gm[:].rearrange("p (b ri k) -> p b ri k", b=BLOC, ri=2, k=M)[:, :, :, 1:M]
            w256a = spool.tile([COUT, BLOC * 64], F32, tag="w256a")
            w256b = spool.tile([COUT, BLOC * 64], F32, tag="w256b")
            wa = w256a[:].rearrange("p (b ri k) -> p b ri k", b=4, ri=2, k=M - 1)
            wb = w256b[:].rearrange("p (b ri k) -> p b ri k", b=4, ri=2, k=M - 1)
            TS(wa, Gall, 2.0, 0.0, OP.mult, OP.add)
            TT(wb, Zall, wa, OP.add)
            TT(wa, Zall, wb, OP.mult)
            for b in range(BLOC):
                nc.vector.tensor_reduce(A12[:, b:b + 1], w256a[:, 64 * b:64 * (b + 1)],
                                        mybir.AxisListType.X, OP.add)

            # vectorized S1/S2 assembly over the 4 batches
            Zr04 = Zsb[:, 0:4]                                  # Zr[k=0] per b
            u4 = Gm[:].rearrange("p (b x) -> p b x", b=BLOC, x=KC)[:, :, 0]
            v4 = spool.tile([COUT, BLOC], F32, tag="v4")
            s2c = spool.tile([COUT, BLOC], F32, tag="s2c")
            w1 = spool.tile([COUT, BLOC], F32, tag="w1")
            TT(v4[:], Zr04, u4, OP.add)
            TT(v4[:], v4[:], e4_s[:], OP.add)                   # v = Zr0+u+e
            TT(s2c[:], Zr04, Zr04, OP.mult)                     # Zr0^2
            TS(w1[:], A12[:], 2.0, 0.0, OP.mult, OP.add)
            TT(s2c[:], s2c[:], w1[:], OP.add)
            TS(w1[:], q4[:], 1.0 / L, 0.0, OP.mult, OP.add)
            TT(s2c[:], s2c[:], w1[:], OP.add)
            TT(w1[:], e4_s[:], v4[:], OP.mult)
            TS(w1[:], w1[:], 2.0, 0.0, OP.mult, OP.add)
            TT(s2c[:], s2c[:], w1[:], OP.add)
            TT(w1[:], e4_s[:], e4_s[:], OP.mult)
            TT(s2c[:], s2c[:], w1[:], OP.subtract)
            TT(w1[:], Zr04, u4, OP.mult)
            TS(w1[:], w1[:], 2.0, 0.0, OP.mult, OP.add)
            TT(s2c[:], s2c[:], w1[:], OP.add)

            stat_in = spool.tile([COUT, 2], F32, tag="stin")
            nc.vector.tensor_reduce(stat_in[:, 0:1], v4[:], mybir.AxisListType.X, OP.add)
            nc.vector.tensor_reduce(stat_in[:, 1:2], s2c[:], mybir.AxisListType.X, OP.add)

            # ---- AllReduce of (64,2) stats across the 8 cores ----
            # (gpsimd ring carries ONLY this traffic, so it fires immediately)
            din = dpool.tile([COUT, 2], F32)
            dout = dpool.tile([COUT, 2], F32)
            nc.gpsimd.dma_start(din[:], stat_in[:])
            if USE_AR:
                nc.gpsimd.collective_compute(
                    "AllReduce", OP.add,
                    replica_groups=[list(range(NCORES))],
                    ins=[din.opt()], outs=[dout.opt()],
                )
            else:
                nc.gpsimd.dma_start(dout[:], din[:])
            st128 = spool.tile([128, 2], F32, tag="st128")
            nc.gpsimd.dma_start(st128[0:64, :], dout[:])
            nc.gpsimd.dma_start(st128[64:128, :], dout[:])

            # ---- Z transpose (per batch, DC row folded into bias later) ----
            # zeb[b]: rows 0:64 = Z^T (modes k=1..32, re|im), rows 64:128 = E_b
            Zflat = spool.tile([COUT, 4 * 64], F32, tag="zflat")  # [b][ri,k>=1]
            nc.vector.tensor_copy(
                Zflat[:].rearrange("p (b ri k) -> p b ri k", b=4, ri=2, k=M - 1),
                Zsb[:].rearrange("p (ri k b) -> p b ri k", ri=2, k=M, b=4)[:, :, :, 1:M])
            for b in range(BLOC):
                tp = psS.tile([CIN, COUT], F32, tag="small")
                nc.tensor.transpose(tp[:], Zflat[:, 64 * b:64 * (b + 1)], id_s[:])
                nc.vector.tensor_copy(zeb[0:64, 64 * b:64 * (b + 1)], tp[:])

            # ---- phase C: single K=128 matmul per tile: [Z;E]^T @ [ABt;xT] ----
            OUT = []
            for j in range(2):
                outj = opool.tile([128, L], BF16, tag=f"out{j}")
                OUT.append(outj)

            # Zr0 (DC) pair-stacking via DRAM bounce (folded into normalize bias)
            zr0d = dpool.tile([COUT, BLOC], F32)
            dma(zr0d[:], Zsb[:, 0:4])
            zr0p = spool.tile([128, 2], F32, tag="zr0p")
            for j in range(2):
                dma(zr0p[0:64, j:j + 1], zr0d[:, 2 * j:2 * j + 1])
                dma(zr0p[64:128, j:j + 1], zr0d[:, 2 * j + 1:2 * j + 2])

            NSTEP = 512
            for j in range(2):
                b0, b1 = 2 * j, 2 * j + 1
                for n in range(L // NSTEP):
                    ps = psB.tile([128, NSTEP], F32, tag="invres")
                    sl = slice(NSTEP * n, NSTEP * (n + 1))
                    nc.tensor.matmul(ps[0:64, :], zeb[:, 64 * b0:64 * b0 + 64],
                                     xtc[b0][:, sl], start=True, stop=True)
                    nc.tensor.matmul(ps[64:128, :], zeb[:, 64 * b1:64 * b1 + 64],
                                     xtc[b1][:, sl], start=True, stop=True,
                                     tile_position=(0, 64))
                    if n % 2 == 0:
                        nc.scalar.copy(OUT[j][:, sl], ps[:])
                    else:
                        nc.vector.tensor_copy(OUT[j][:, sl], ps[:])

            # ---- BN scale/shift from all-reduced stats ----
            mean = spool.tile([128, 1], F32, tag="mean")
            ex2 = spool.tile([128, 1], F32, tag="ex2")
            var = spool.tile([128, 1], F32, tag="var")
            sv = spool.tile([128, 1], F32, tag="sv")
            sh = spool.tile([128, 1], F32, tag="sh")
            wk = spool.tile([128, 1], F32, tag="wk")
            TS(mean[:], st128[:, 0:1], 1.0 / B, 0.0, OP.mult, OP.add)
            TS(ex2[:], st128[:, 1:2], 1.0 / B, 0.0, OP.mult, OP.add)
            TT(wk[:], mean[:], mean[:], OP.mult)
            TT(var[:], ex2[:], wk[:], OP.subtract)
            TS(var[:], var[:], 1.0, EPS, OP.mult, OP.add)
            nc.scalar.activation(wk[:], var[:], mybir.ActivationFunctionType.Sqrt)
            nc.vector.reciprocal(sv[:], wk[:])
            TT(sv[:], sv[:], bnp_s[:, 0:1], OP.mult)            # s = bn_scale/std
            TT(wk[:], mean[:], sv[:], OP.mult)
            TT(sh[:], bnp_s[:, 1:2], wk[:], OP.subtract)        # shift = bias - mean*s

            bjs = []
            for j in range(2):
                bj = spool.tile([128, 1], F32, tag=f"bj{j}")
                TT(bj[:], ep_s[:, j:j + 1], zr0p[:, j:j + 1], OP.add)
                TT(bj[:], bj[:], sv[:], OP.mult)                # s*(e_b + Zr0)
                TT(bj[:], bj[:], sh[:], OP.add)                 # + shift
                bjs.append(bj)
            # 16 units; vector is ~2x faster per unit than scalar -> 10/6 split
            NQ = L // 8
            units = [(n2, j) for n2 in range(8) for j in range(2)]
            for i, (n2, j) in enumerate(units):
                q = slice(n2 * NQ, (n2 + 1) * NQ)
                od = out_d[2 * j:2 * j + 2].rearrange("a b l -> (a b) l")
                if i % 8 < 3:        # 6 of 16 on scalar
                    nc.scalar.activation(OUT[j][:, q], OUT[j][:, q],
                                         mybir.ActivationFunctionType.Relu,
                                         bias=bjs[j][:], scale=sv[:])
                    nc.scalar.dma_start(od[:, q], OUT[j][:, q])
                else:                # 10 of 16 on vector
                    TS(OUT[j][:, q], OUT[j][:, q], sv[:], bjs[j][:],
                       OP.mult, OP.add)
                    TS(OUT[j][:, q], OUT[j][:, q], 0.0, 0.0, OP.max, OP.add)
                    dma(od[:, q], OUT[j][:, q])

    nc.compile()
    return nc


_NC_CACHE = {}


def _get_nc():
    if "nc" not in _NC_CACHE:
        _NC_CACHE["nc"] = _build()
    return _NC_CACHE["nc"]


def _host_prep(x, t_emb, spec_w_real, spec_w_imag, dense_re, dense_im,
               conv_kernel, conv_bias, tc_weights, psi_kernel, bn_scale, bn_bias):
    """Build per-core input maps (small tensors precomputed on host)."""
    k = np.arange(M)
    l = np.arange(L)
    ang = 2.0 * np.pi * np.outer(l, k) / L
    CSt = np.concatenate([np.cos(ang) / L, -np.sin(ang) / L], axis=1)   # (L, 66)
    angk = ang[:, 1:]                                # drop DC mode
    ABt = np.concatenate([(2.0 * np.cos(angk)).T,
                          (-2.0 * np.sin(angk)).T], axis=0).astype(np.float32)

    tr = (t_emb @ dense_re).astype(np.float32)      # (B, 33)
    ti = (t_emb @ dense_im).astype(np.float32)
    psi = (t_emb @ psi_kernel).astype(np.float32)
    w_t, b_t = psi[:, :COUT], psi[:, COUT:]
    E = np.einsum("ij,bj,oj->bio", conv_kernel, w_t, tc_weights).astype(np.float32)
    e = ((conv_bias * w_t) @ tc_weights.T + b_t).astype(np.float32)      # (B, 64)

    Wcat = np.concatenate([spec_w_real, spec_w_imag], axis=2)            # (33, 64, 128)
    wm = np.ascontiguousarray(Wcat.transpose(1, 0, 2).reshape(CIN, M * 128)).astype(NP_BF16)
    cst = np.ascontiguousarray(
        CSt.reshape(NCHUNK, 128, KC).transpose(1, 0, 2).reshape(128, NCHUNK * KC)
    ).astype(NP_BF16)
    abt = ABt.astype(NP_BF16)
    idm = np.eye(64, dtype=np.float32)
    bnp = np.stack([np.tile(bn_scale, 2), np.tile(bn_bias, 2)], axis=1).astype(np.float32)

    x32 = x.astype(np.float32)
    in_maps = []
    for c in range(NCORES):
        sl = slice(BLOC * c, BLOC * (c + 1))
        xs = x32[sl]                                             # (4, L, 64)
        xb = np.ascontiguousarray(
            xs.reshape(BLOC, NCHUNK, 128, CIN).transpose(0, 2, 1, 3)
            .reshape(BLOC, 128, NCHUNK * CIN)).astype(NP_BF16)
        xt = np.ascontiguousarray(xs.transpose(0, 2, 1)).astype(NP_BF16)
        trc, tic = tr[sl], ti[sl]                                # (4, 33)
        tmod = np.concatenate([
            trc.T.reshape(-1), tic.T.reshape(-1)                 # [4k+b] each
        ]).astype(np.float32)
        tm = np.broadcast_to(tmod, (COUT, 2 * 4 * M)).copy()
        Ec = E[sl]                                               # (4, 64, 64)
        ec = e[sl]                                               # (4, 64)
        ep = np.stack([
            np.concatenate([ec[0], ec[1]]),
            np.concatenate([ec[2], ec[3]]),
        ], axis=1).astype(np.float32)                            # (128, 2)
        Ecat = np.ascontiguousarray(Ec.transpose(1, 0, 2).reshape(CIN, BLOC * COUT))
        in_maps.append({
            "xb": xb,
            "cst": cst,
            "xt": xt,
            "abt": abt,
            "wm": wm,
            "ebf": Ecat.astype(NP_BF16),
            "ef": Ecat.astype(np.float32),
            "tm": tm,
            "e4": np.ascontiguousarray(ec.T).astype(np.float32),
            "ep": ep,
            "bnp": bnp,
            "idm": idm,
        })
    return in_maps


def kernel(**inputs):
    inputs = {k: np.asarray(v) for k, v in inputs.items()}
    nc = _get_nc()
    in_maps = _host_prep(**inputs)
    res = bass_utils.run_bass_kernel_spmd(
        nc, in_maps, core_ids=list(range(NCORES)),
        trace=bool(int(os.environ.get("KBENCH_TRACE", "0"))),
    )
    out = np.empty((B, L, COUT), np.float32)
    for c in range(NCORES):
        o = res.results[c]["out"].astype(np.float32)     # (4, 64, L)
        out[BLOC * c:BLOC * (c + 1)] = np.ascontiguousarray(o.transpose(0, 2, 1))
    _NC_CACHE["last_results"] = res
    return out


# revision 8
# speedup vs baseline: 1.3029x; 1.0216x over previous
"""Trainium2 Bass kernel for CTUNOBlock1D (spectral conv + time conv + batchnorm + relu).

Strategy (data-parallel over batch, 8 cores, 4 batches/core):
  - rfft/irfft use only 33 modes -> implemented as DFT matmuls against small
    trig tables (bf16, f32 accumulation on PE).
  - phase A computes the forward DFT and the Gram matrix x^T x per batch,
    sharing each x chunk as the stationary operand of two back-to-back
    matmuls (DFT table rhs, then the chunk itself).
  - BN statistics are computed exactly in mode space (Parseval) right after
    the mode modulation, so the tiny (64,2) stats AllReduce is triggered as
    early as possible and overlaps the inverse/residual phase (the gpsimd
    ring carries only the AR traffic).
  - residual branch is folded on host: E_b = K diag(w_t) Wt^T, e_b bias; the
    device computes out^T = Z^T @ ABt + E_b^T @ x^T per batch via one K=128
    matmul per tile against [ABt; x^T] tiles (ABt replicated SBUF->SBUF on
    the scalar ring), with PSUM->SBUF copies alternating scalar/vector.
  - BN scale/shift (+ folded e_b) + ReLU applied in 16 units split between
    the vector (10) and scalar (6) engines, each unit's output DMA issued
    from an engine-local ring.
  - output is written channel-major and transposed on host.
"""

import os
import numpy as np

import concourse.bass as bass
import concourse.mybir as mybir
import concourse.bacc as bacc
import concourse.tile as tile
from concourse import bass_utils

F32 = mybir.dt.float32
BF16 = mybir.dt.bfloat16
NP_BF16 = mybir.dt.np(BF16)

B, L, CIN, COUT, TEMB = 32, 8192, 64, 64, 256
M = 33            # retained rfft modes
KC = 2 * M        # 66 (real|imag concat)
NCORES = 8
BLOC = B // NCORES   # 4 batches per core
EPS = 1e-5
NCHUNK = L // 128    # 64 l-chunks of 128
USE_AR = bool(int(os.environ.get("KBENCH_AR", "1")))


def _build():
    nc = bacc.Bacc(None, target_bir_lowering=False)

    xb_d = nc.dram_tensor("xb", [BLOC, 128, NCHUNK * CIN], BF16, kind="ExternalInput")
    cst_d = nc.dram_tensor("cst", [128, NCHUNK * KC], BF16, kind="ExternalInput")
    xt_d = nc.dram_tensor("xt", [BLOC, CIN, L], BF16, kind="ExternalInput")
    abt_d = nc.dram_tensor("abt", [CIN, L], BF16, kind="ExternalInput")
    wm_d = nc.dram_tensor("wm", [CIN, M * 128], BF16, kind="ExternalInput")
    ebf_d = nc.dram_tensor("ebf", [CIN, BLOC * COUT], BF16, kind="ExternalInput")
    ef_d = nc.dram_tensor("ef", [CIN, BLOC * COUT], F32, kind="ExternalInput")
    tm_d = nc.dram_tensor("tm", [COUT, 2 * 4 * M], F32, kind="ExternalInput")
    e4_d = nc.dram_tensor("e4", [COUT, BLOC], F32, kind="ExternalInput")
    ep_d = nc.dram_tensor("ep", [128, 2], F32, kind="ExternalInput")
    bnp_d = nc.dram_tensor("bnp", [128, 2], F32, kind="ExternalInput")
    id_d = nc.dram_tensor("idm", [64, 64], F32, kind="ExternalInput")
    out_d = nc.dram_tensor("out", [BLOC, COUT, L], BF16, kind="ExternalOutput")

    with tile.TileContext(nc) as tc:
        with (
            tc.tile_pool(name="const", bufs=1) as cpool,
            tc.tile_pool(name="xs", bufs=1) as xpool,
            tc.tile_pool(name="xtp", bufs=1) as xtpool,
            tc.tile_pool(name="outb", bufs=1) as opool,
            tc.tile_pool(name="small", bufs=2) as spool,
            tc.tile_pool(name="psA", bufs=1, space=bass.MemorySpace.PSUM) as psA,
            tc.tile_pool(name="psS", bufs=2, space=bass.MemorySpace.PSUM) as psS,
            tc.tile_pool(name="psB", bufs=2, space=bass.MemorySpace.PSUM) as psB,
            tc.tile_pool(name="dram", bufs=1, space=bass.MemorySpace.DRAM) as dpool,
        ):
            dma = nc.sync.dma_start

            # ---- phase-A critical loads first, then phase-C bulk ----
            cst_s = cpool.tile([128, NCHUNK * KC], BF16)
            dma(cst_s[:], cst_d[:])
            xss = []
            for b in range(BLOC):
                xs = xpool.tile([128, NCHUNK * CIN], BF16, tag=f"xs{b}")
                dma(xs[:], xb_d[b])
                xss.append(xs)

            # small constants (phase B/stats path)
            wm_s = cpool.tile([CIN, M * 128], BF16)
            tm_s = cpool.tile([COUT, 2 * 4 * M], F32)
            e4_s = cpool.tile([COUT, BLOC], F32)
            ep_s = cpool.tile([128, 2], F32)
            bnp_s = cpool.tile([128, 2], F32)
            id_s = cpool.tile([64, 64], F32)
            ones_s = cpool.tile([64, 1], F32)
            dma(wm_s[:], wm_d[:])
            dma(tm_s[:], tm_d[:])
            dma(e4_s[:], e4_d[:])
            dma(ep_s[:], ep_d[:])
            dma(bnp_s[:], bnp_d[:])
            dma(id_s[:], id_d[:])
            nc.vector.memset(ones_s[:], 1.0)

            ebf_s = cpool.tile([CIN, BLOC * COUT], BF16)   # [i, 64b+o]
            ef_s = cpool.tile([CIN, BLOC * COUT], F32)
            zeb = cpool.tile([128, BLOC * COUT], BF16)     # [0:64]=Z^T, [64:128]=E
            dma(ebf_s[:], ebf_d[:])
            dma(ef_s[:], ef_d[:])
            dma(zeb[64:128, :], ebf_d[:])

            # phase-C bulk: [ABt; x^T] per batch; ABt lands in xtc[0] and is
            # replicated SBUF->SBUF into the other tops on the scalar ring
            xtc = []
            LQ = L // 4
            for b in range(BLOC):
                xt = xtpool.tile([128, L], BF16, tag=f"xtc{b}")
                xtc.append(xt)
            # quarter-granular loads so phase C streams behind the DMA
            for qq in range(4):
                ql = slice(LQ * qq, LQ * (qq + 1))
                dma(xtc[0][0:64, ql], abt_d[:, ql])
                for b in range(BLOC):
                    dma(xtc[b][64:128, ql], xt_d[b][:, ql])
            # ABt replication on the vector engine (keeps scalar ring free
            # for the stats-path copies that feed the AR trigger)
            for qq in range(4):
                ql = slice(LQ * qq, LQ * (qq + 1))
                for b in range(1, BLOC):
                    nc.vector.tensor_copy(xtc[b][0:64, ql], xtc[0][0:64, ql])

            # early dummy Sqrt to pre-load the ACT table set
            warm = spool.tile([1, 1], F32)
            nc.vector.memset(warm[:], 1.0)
            nc.scalar.activation(warm[:], warm[:], mybir.ActivationFunctionType.Sqrt)

            # ---- phase A: forward DFT + Gram per batch ----
            Xsb = cpool.tile([CIN, BLOC * KC], BF16)    # [c, 66b+kcat]
            Gsb = cpool.tile([CIN, BLOC * CIN], BF16)   # [c, 64b+c']
            for b in range(BLOC):
                xs = xss[b]
                xacc = psA.tile([CIN, KC], F32, tag="xacc")
                gacc = psA.tile([CIN, CIN], F32, tag="gacc")
                for u in range(NCHUNK):
                    lhs = xs[:, CIN * u:CIN * (u + 1)]
                    nc.tensor.matmul(xacc[:], lhs, cst_s[:, KC * u:KC * (u + 1)],
                                     start=(u == 0), stop=(u == NCHUNK - 1))
                    nc.tensor.matmul(gacc[:], lhs, lhs,
                                     start=(u == 0), stop=(u == NCHUNK - 1))
                nc.vector.tensor_copy(Xsb[:, KC * b:KC * (b + 1)], xacc[:])
                nc.vector.tensor_copy(Gsb[:, CIN * b:CIN * (b + 1)], gacc[:])

            # ---- phase B: mode mixing ----
            # P12[0:64, 8k+x] = Wr[k]^T @ X_x ; P12[64:128] likewise for Wi
            # (x = 4j+b, j=0: Xr, 1: Xi)
            P12 = psS.tile([128, M * 8], F32, tag="p12")
            Xr4 = Xsb[:].rearrange("p (b j k) -> p j b k", b=BLOC, j=2, k=M)
            for k in range(M):
                nc.tensor.matmul(P12[:, 8 * k:8 * (k + 1)],
                                 wm_s[:, 128 * k:128 * (k + 1)],
                                 Xr4[:, :, :, k], start=True, stop=True)

            TT = nc.vector.tensor_tensor
            TS = nc.vector.tensor_scalar
            OP = mybir.AluOpType
            Psb = spool.tile([COUT, 2 * M * 8], F32, tag="psb")
            nc.scalar.copy(Psb[:, 0:M * 8], P12[0:64, :])
            nc.scalar.copy(Psb[:, M * 8:2 * M * 8], P12[64:128, :])
            Yr = spool.tile([COUT, 4 * M], F32, tag="yr")   # [(k,b)] = 4k+b
            Yi = spool.tile([COUT, 4 * M], F32, tag="yi")
            Pk1 = Psb[:, 0:M * 8].rearrange("p (k x) -> p k x", k=M, x=8)
            Pk2 = Psb[:, M * 8:2 * M * 8].rearrange("p (k x) -> p k x", k=M, x=8)
            Yrv = Yr[:].rearrange("p (k b) -> p k b", k=M, b=4)
            Yiv = Yi[:].rearrange("p (k b) -> p k b", k=M, b=4)
            TT(Yrv, Pk1[:, :, 0:4], Pk2[:, :, 4:8], OP.subtract)
            TT(Yiv, Pk2[:, :, 0:4], Pk1[:, :, 4:8], OP.add)

            Zsb = cpool.tile([COUT, 2 * 4 * M], F32)  # [(ri,k,b)] = 132ri+4k+b
            t1 = spool.tile([COUT, 4 * M], F32, tag="t1")
            t2 = spool.tile([COUT, 4 * M], F32, tag="t2")
            ntm = 4 * M
            TT(t1[:], Yr[:], tm_s[:, 0:ntm], OP.mult)
            TT(t2[:], Yi[:], tm_s[:, ntm:2 * ntm], OP.mult)
            TT(Zsb[:, 0:ntm], t1[:], t2[:], OP.subtract)
            TT(t1[:], Yi[:], tm_s[:, 0:ntm], OP.mult)
            TT(t2[:], Yr[:], tm_s[:, ntm:2 * ntm], OP.mult)
            TT(Zsb[:, ntm:2 * ntm], t1[:], t2[:], OP.add)

            # ---- stats in mode space, FIRST (AR trigger asap) ----
            Gm = cpool.tile([COUT, BLOC * KC], F32)     # [o, 66b+33ri+k]
            q4 = spool.tile([COUT, BLOC], F32, tag="q4")
            A12 = spool.tile([COUT, BLOC], F32, tag="a12")
            gmp = psS.tile([COUT, BLOC * KC], F32, tag="small")
            m1p = psS.tile([CIN, BLOC * COUT], F32, tag="small")
            for b in range(BLOC):
                nc.tensor.matmul(gmp[:, KC * b:KC * (b + 1)],
                                 ebf_s[:, 64 * b:64 * (b + 1)],
                                 Xsb[:, KC * b:KC * (b + 1)], start=True, stop=True)
                nc.tensor.matmul(m1p[:, 64 * b:64 * (b + 1)],
                                 Gsb[:, 64 * b:64 * (b + 1)],
                                 ebf_s[:, 64 * b:64 * (b + 1)], start=True, stop=True)
            nc.scalar.copy(Gm[:], gmp[:])
            em = spool.tile([CIN, BLOC * COUT], F32, tag="em")
            TT(em[:], m1p[:], ef_s[:], OP.mult)
            qp = psS.tile([COUT, BLOC], F32, tag="small")
            for b in range(BLOC):
                nc.tensor.matmul(qp[:, b:b + 1], em[:, 64 * b:64 * (b + 1)],
                                 ones_s[:], start=True, stop=True)
            nc.vector.tensor_copy(q4[:], qp[:])
            # A12 = sum_k>=1 Zr*(Zr+2Gr) + Zi*(Zi+2Gi)  (= A1 + 2*A2)
            Zall = Zsb[:].rearrange("p (ri k b) -> p b ri k", ri=2, k=M, b=4)[:, :, :, 1:M]
            Gall = Gm[:].rearrange("p (b ri k) -> p b ri k", b=BLOC, ri=2, k=M)[:, :, :, 1:M]
            w256a = spool.tile([COUT, BLOC * 64], F32, tag="w256a")
            w256b = spool.tile([COUT, BLOC * 64], F32, tag="w256b")
            wa = w256a[:].rearrange("p (b ri k) -> p b ri k", b=4, ri=2, k=M - 1)
            wb = w256b[:].rearrange("p (b ri k) -> p b ri k", b=4, ri=2, k=M - 1)
            TS(wa, Gall, 2.0, 0.0, OP.mult, OP.add)
            TT(wb, Zall, wa, OP.add)
            TT(wa, Zall, wb, OP.mult)
            for b in range(BLOC):
                nc.vector.tensor_reduce(A12[:, b:b + 1], w256a[:, 64 * b:64 * (b + 1)],
                                        mybir.AxisListType.X, OP.add)

            # vectorized S1/S2 assembly over the 4 batches
            Zr04 = Zsb[:, 0:4]                                  # Zr[k=0] per b
            u4 = Gm[:].rearrange("p (b x) -> p b x", b=BLOC, x=KC)[:, :, 0]
            v4 = spool.tile([COUT, BLOC], F32, tag="v4")
            s2c = spool.tile([COUT, BLOC], F32, tag="s2c")
            w1 = spool.tile([COUT, BLOC], F32, tag="w1")
            TT(v4[:], Zr04, u4, OP.add)
            TT(v4[:], v4[:], e4_s[:], OP.add)                   # v = Zr0+u+e
            TT(s2c[:], Zr04, Zr04, OP.mult)                     # Zr0^2
            TS(w1[:], A12[:], 2.0, 0.0, OP.mult, OP.add)
            TT(s2c[:], s2c[:], w1[:], OP.add)
            TS(w1[:], q4[:], 1.0 / L, 0.0, OP.mult, OP.add)
            TT(s2c[:], s2c[:], w1[:], OP.add)
            TT(w1[:], e4_s[:], v4[:], OP.mult)
            TS(w1[:], w1[:], 2.0, 0.0, OP.mult, OP.add)
            TT(s2c[:], s2c[:], w1[:], OP.add)
            TT(w1[:], e4_s[:], e4_s[:], OP.mult)
            TT(s2c[:], s2c[:], w1[:], OP.subtract)
            TT(w1[:], Zr04, u4, OP.mult)
            TS(w1[:], w1[:], 2.0, 0.0, OP.mult, OP.add)
            TT(s2c[:], s2c[:], w1[:], OP.add)

            stat_in = spool.tile([COUT, 2], F32, tag="stin")
            nc.vector.tensor_reduce(stat_in[:, 0:1], v4[:], mybir.AxisListType.X, OP.add)
            nc.vector.tensor_reduce(stat_in[:, 1:2], s2c[:], mybir.AxisListType.X, OP.add)

            # ---- AllReduce of (64,2) stats across the 8 cores ----
            # (gpsimd ring carries ONLY this traffic, so it fires immediately)
            din = dpool.tile([COUT, 2], F32)
            dout = dpool.tile([COUT, 2], F32)
            nc.gpsimd.dma_start(din[:], stat_in[:])
            if USE_AR:
                nc.gpsimd.collective_compute(
                    "AllReduce", OP.add,
                    replica_groups=[list(range(NCORES))],
                    ins=[din.opt()], outs=[dout.opt()],
                )
            else:
                nc.gpsimd.dma_start(dout[:], din[:])
            st128 = spool.tile([128, 2], F32, tag="st128")
            nc.gpsimd.dma_start(st128[0:64, :], dout[:])
            nc.gpsimd.dma_start(st128[64:128, :], dout[:])

            # ---- Z transpose (per batch, DC row folded into bias later) ----
            # zeb[b]: rows 0:64 = Z^T (modes k=1..32, re|im), rows 64:128 = E_b
            Zflat = spool.tile([COUT, 4 * 64], F32, tag="zflat")  # [b][ri,k>=1]
            nc.vector.tensor_copy(
                Zflat[:].rearrange("p (b ri k) -> p b ri k", b=4, ri=2, k=M - 1),
                Zsb[:].rearrange("p (ri k b) -> p b ri k", ri=2, k=M, b=4)[:, :, :, 1:M])
            for b in range(BLOC):
                tp = psS.tile([CIN, COUT], F32, tag="small")
                nc.tensor.transpose(tp[:], Zflat[:, 64 * b:64 * (b + 1)], id_s[:])
                nc.vector.tensor_copy(zeb[0:64, 64 * b:64 * (b + 1)], tp[:])

            # ---- phase C: single K=128 matmul per tile: [Z;E]^T @ [ABt;xT] ----
            OUT = []
            for j in range(2):
                outj = opool.tile([128, L], BF16, tag=f"out{j}")
                OUT.append(outj)

            # Zr0 (DC) pair-stacking via DRAM bounce (folded into normalize bias)
            zr0d = dpool.tile([COUT, BLOC], F32)
            dma(zr0d[:], Zsb[:, 0:4])
            zr0p = spool.tile([128, 2], F32, tag="zr0p")
            for j in range(2):
                dma(zr0p[0:64, j:j + 1], zr0d[:, 2 * j:2 * j + 1])
                dma(zr0p[64:128, j:j + 1], zr0d[:, 2 * j + 1:2 * j + 2])

            NSTEP = 512
            for j in range(2):
                b0, b1 = 2 * j, 2 * j + 1
                for n in range(L // NSTEP):
                    ps = psB.tile([128, NSTEP], F32, tag="invres")
                    sl = slice(NSTEP * n, NSTEP * (n + 1))
                    nc.tensor.matmul(ps[0:64, :], zeb[:, 64 * b0:64 * b0 + 64],
                                     xtc[b0][:, sl], start=True, stop=True)
                    nc.tensor.matmul(ps[64:128, :], zeb[:, 64 * b1:64 * b1 + 64],
                                     xtc[b1][:, sl], start=True, stop=True,
                                     tile_position=(0, 64))
                    if n % 2 == 0:
                        nc.scalar.copy(OUT[j][:, sl], ps[:])
                    else:
                        nc.vector.tensor_copy(OUT[j][:, sl], ps[:])

            # ---- BN scale/shift from all-reduced stats ----
            mean = spool.tile([128, 1], F32, tag="mean")
            ex2 = spool.tile([128, 1], F32, tag="ex2")
            var = spool.tile([128, 1], F32, tag="var")
            sv = spool.tile([128, 1], F32, tag="sv")
            sh = spool.tile([128, 1], F32, tag="sh")
            wk = spool.tile([128, 1], F32, tag="wk")
            TS(mean[:], st128[:, 0:1], 1.0 / B, 0.0, OP.mult, OP.add)
            TS(ex2[:], st128[:, 1:2], 1.0 / B, 0.0, OP.mult, OP.add)
            TT(wk[:], mean[:], mean[:], OP.mult)
            TT(var[:], ex2[:], wk[:], OP.subtract)
            TS(var[:], var[:], 1.0, EPS, OP.mult, OP.add)
            nc.scalar.activation(wk[:], var[:], mybir.ActivationFunctionType.Sqrt)
            nc.vector.reciprocal(sv[:], wk[:])
            TT(sv[:], sv[:], bnp_s[:, 0:1], OP.mult)            # s = bn_scale/std
            TT(wk[:], mean[:], sv[:], OP.mult)
            TT(sh[:], bnp_s[:, 1:2], wk[:], OP.subtract)        # shift = bias - mean*s

            bjs = []
            for j in range(2):
                bj = spool.tile([128, 1], F32, tag=f"bj{j}")
                TT(bj[:], ep_s[:, j:j + 1], zr0p[:, j:j + 1], OP.add)
                TT(bj[:], bj[:], sv[:], OP.mult)                # s*(e_b + Zr0)
                TT(bj[:], bj[:], sh[:], OP.add)                 # + shift
                bjs.append(bj)
            # 16 units; vector is ~2x faster per unit than scalar -> 10/6 split
            NQ = L // 8
            units = [(n2, j) for n2 in range(8) for j in range(2)]
            for i, (n2, j) in enumerate(units):
                q = slice(n2 * NQ, (n2 + 1) * NQ)
                od = out_d[2 * j:2 * j + 2].rearrange("a b l -> (a b) l")
                if i % 8 < 3:        # 6 of 16 on scalar
                    nc.scalar.activation(OUT[j][:, q], OUT[j][:, q],
                                         mybir.ActivationFunctionType.Relu,
                                         bias=bjs[j][:], scale=sv[:])
                    nc.scalar.dma_start(od[:, q], OUT[j][:, q])
                else:                # 10 of 16 on vector
                    TS(OUT[j][:, q], OUT[j][:, q], sv[:], bjs[j][:],
                       OP.mult, OP.add)
                    TS(OUT[j][:, q], OUT[j][:, q], 0.0, 0.0, OP.max, OP.add)
                    dma(od[:, q], OUT[j][:, q])

    nc.compile()
    return nc


_NC_CACHE = {}


def _get_nc():
    if "nc" not in _NC_CACHE:
        _NC_CACHE["nc"] = _build()
    return _NC_CACHE["nc"]


def _host_prep(x, t_emb, spec_w_real, spec_w_imag, dense_re, dense_im,
               conv_kernel, conv_bias, tc_weights, psi_kernel, bn_scale, bn_bias):
    """Build per-core input maps (small tensors precomputed on host)."""
    k = np.arange(M)
    l = np.arange(L)
    ang = 2.0 * np.pi * np.outer(l, k) / L
    CSt = np.concatenate([np.cos(ang) / L, -np.sin(ang) / L], axis=1)   # (L, 66)
    angk = ang[:, 1:]                                # drop DC mode
    ABt = np.concatenate([(2.0 * np.cos(angk)).T,
                          (-2.0 * np.sin(angk)).T], axis=0).astype(np.float32)

    tr = (t_emb @ dense_re).astype(np.float32)      # (B, 33)
    ti = (t_emb @ dense_im).astype(np.float32)
    psi = (t_emb @ psi_kernel).astype(np.float32)
    w_t, b_t = psi[:, :COUT], psi[:, COUT:]
    E = np.einsum("ij,bj,oj->bio", conv_kernel, w_t, tc_weights).astype(np.float32)
    e = ((conv_bias * w_t) @ tc_weights.T + b_t).astype(np.float32)      # (B, 64)

    Wcat = np.concatenate([spec_w_real, spec_w_imag], axis=2)            # (33, 64, 128)
    wm = np.ascontiguousarray(Wcat.transpose(1, 0, 2).reshape(CIN, M * 128)).astype(NP_BF16)
    cst = np.ascontiguousarray(
        CSt.reshape(NCHUNK, 128, KC).transpose(1, 0, 2).reshape(128, NCHUNK * KC)
    ).astype(NP_BF16)
    abt = ABt.astype(NP_BF16)
    idm = np.eye(64, dtype=np.float32)
    bnp = np.stack([np.tile(bn_scale, 2), np.tile(bn_bias, 2)], axis=1).astype(np.float32)

    x32 = x.astype(np.float32)
    in_maps = []
    for c in range(NCORES):
        sl = slice(BLOC * c, BLOC * (c + 1))
        xs = x32[sl]                                             # (4, L, 64)
        xb = np.ascontiguousarray(
            xs.reshape(BLOC, NCHUNK, 128, CIN).transpose(0, 2, 1, 3)
            .reshape(BLOC, 128, NCHUNK * CIN)).astype(NP_BF16)
        xt = np.ascontiguousarray(xs.transpose(0, 2, 1)).astype(NP_BF16)
        trc, tic = tr[sl], ti[sl]                                # (4, 33)
        tmod = np.concatenate([
            trc.T.reshape(-1), tic.T.reshape(-1)                 # [4k+b] each
        ]).astype(np.float32)
        tm = np.broadcast_to(tmod, (COUT, 2 * 4 * M)).copy()
        Ec = E[sl]                                               # (4, 64, 64)
        ec = e[sl]                                               # (4, 64)
        ep = np.stack([
            np.concatenate([ec[0], ec[1]]),
            np.concatenate([ec[2], ec[3]]),
        ], axis=1).astype(np.float32)                            # (128, 2)
        Ecat = np.ascontiguousarray(Ec.transpose(1, 0, 2).reshape(CIN, BLOC * COUT))
        in_maps.append({
            "xb": xb,
            "cst": cst,
            "xt": xt,
            "abt": abt,
            "wm": wm,
            "ebf": Ecat.astype(NP_BF16),
            "ef": Ecat.astype(np.float32),
            "tm": tm,
            "e4": np.ascontiguousarray(ec.T).astype(np.float32),
            "ep": ep,
            "bnp": bnp,
            "idm": idm,
        })
    return in_maps


def kernel(**inputs):
    inputs = {k: np.asarray(v) for k, v in inputs.items()}
    nc = _get_nc()
    in_maps = _host_prep(**inputs)
    res = bass_utils.run_bass_kernel_spmd(
        nc, in_maps, core_ids=list(range(NCORES)),
        trace=bool(int(os.environ.get("KBENCH_TRACE", "0"))),
    )
    out = np.empty((B, L, COUT), np.float32)
    for c in range(NCORES):
        o = res.results[c]["out"].astype(np.float32)     # (4, 64, L)
        out[BLOC * c:BLOC * (c + 1)] = np.ascontiguousarray(o.transpose(0, 2, 1))
    _NC_CACHE["last_results"] = res
    return out
